# revision 11
# baseline (speedup 1.0000x reference)
"""Trainium2 Bass kernel for the LoRA-BC block (nn_LoRABCBlock).

Computation (per reference):
    base = x @ w_base.T
    h = layernorm(x) * gamma + beta
    qkv = h @ w_qkv.T ; attention (2 heads, head_dim 32) over full sequence
    attn_out = attn_output @ w_attn_out.T
    delta = ((h + attn_out) @ lora_down) @ lora_up
    out = base + (1/8) * delta

Sharding: data-parallel over (batch, seq-half) -> 8 cores. Each core owns
1024 query rows of one batch element, computes k/v over that batch's full
2048-row sequence, and produces its 1024 output rows. Weights replicated.
No collectives.

Key structure (v2):
  - All activation/weight transposes go through the DMA XBAR
    (dma_start_transpose), not the PE array.
  - gamma/beta are folded into w_qkv / lora_down (weights) and bias
    columns, so the normalized activation z=(x-mu)*rstd is the only
    transposed LN tensor; the base matmul runs on centered xc=(x-mu)
    with the mu*colsum(w_base) rank-1 term and the rank-8 LoRA update
    fused into a single K=9 matmul.
  - Attention scores are computed transposed (scoresT[j,m]) so softmax
    probabilities come out in the layout attn@v needs; the softmax
    denominator rides along as an appended ones-column of V.
"""

import sys

sys.path.insert(0, "/opt/trn_rl_repo")

from contextlib import ExitStack

import numpy as np

import concourse.bass as bass
import concourse.tile as tile
from concourse import bacc, mybir
from concourse.bass_utils import run_bass_kernel_spmd
from concourse.masks import make_identity

F32 = mybir.dt.float32
BF16 = mybir.dt.bfloat16
AF = mybir.ActivationFunctionType
MUL = mybir.AluOpType.mult

E = 1024          # embed dim
DM = 1024         # d_model
R = 8             # lora rank
SCALING = 1.0 / R
DA = 64           # attn dim
NH = 2            # heads
HD = DA // NH     # head dim = 32
SOWN = 1024       # rows owned per core
SFULL = 2048      # rows per batch element
NC = 8            # cores
P = 128
KT = E // P       # 8 k-tiles
MT = SOWN // P    # 8 own m-tiles
ST = SFULL // P   # 16 sequence tiles
JT = 2 * ST       # 32 head-stacked j'-tiles
ATT_SCALE = float(HD) ** -0.5


def build_kernel():
    nc = bacc.Bacc("TRN2", target_bir_lowering=False, debug=False, num_devices=NC)

    x_own = nc.dram_tensor("x_own", [SOWN, E], F32, kind="ExternalInput").ap()
    x_oth = nc.dram_tensor("x_oth", [SOWN, E], F32, kind="ExternalInput").ap()
    w_base = nc.dram_tensor("w_base", [DM, E], F32, kind="ExternalInput").ap()
    ln_g = nc.dram_tensor("ln_g", [E], F32, kind="ExternalInput").ap()
    ln_b = nc.dram_tensor("ln_b", [E], F32, kind="ExternalInput").ap()
    ld = nc.dram_tensor("ld", [E, R], F32, kind="ExternalInput").ap()
    lu = nc.dram_tensor("lu", [R, DM], F32, kind="ExternalInput").ap()
    w_qkv = nc.dram_tensor("w_qkv", [3 * DA, E], F32, kind="ExternalInput").ap()
    w_ao = nc.dram_tensor("w_ao", [E, DA], F32, kind="ExternalInput").ap()
    out_d = nc.dram_tensor("out", [SOWN, DM], F32, kind="ExternalOutput").ap()

    with tile.TileContext(nc) as tc, ExitStack() as ctx:
        persist = ctx.enter_context(tc.tile_pool(name="persist", bufs=1))
        ldp = ctx.enter_context(tc.tile_pool(name="loads", bufs=3))
        zh_pool = ctx.enter_context(tc.tile_pool(name="zh", bufs=3))
        xc_pool = ctx.enter_context(tc.tile_pool(name="xc", bufs=2))
        st_pool = ctx.enter_context(tc.tile_pool(name="stats", bufs=4))
        pt_pool = ctx.enter_context(tc.tile_pool(name="pt", bufs=3))
        o_pool = ctx.enter_context(tc.tile_pool(name="outs", bufs=2))
        ps = ctx.enter_context(tc.tile_pool(name="ps", bufs=1, space="PSUM"))

        _n = [0]

        def ps_tile(shape, tag, bufs):
            _n[0] += 1
            return ps.tile(shape, F32, tag=tag, bufs=bufs,
                           name=f"ps_{tag}_{_n[0]}")

        def big_ps():
            # [128, 1024] fp32 = 2 PSUM banks; matmuls target 512-wide halves
            return ps_tile([P, 1024], "big", 2)

        def av_ps():
            # rows 0:64 = attn@v accum; row 64 / row 96 = per-head softmax
            # denominators (legal start partitions for vector ops)
            return ps_tile([97, 512], "av", 2)

        def mu_ps():
            return ps_tile([DA, 512], "mu", 2)

        # ---------------- phase 0: constants + weights ----------------
        ident = persist.tile([P, P], F32, tag="ident")
        make_identity(nc, ident)
        identh = persist.tile([P, P], BF16, tag="identh")
        nc.vector.tensor_copy(out=identh, in_=ident)
        ones_row = persist.tile([1, DA], BF16, tag="ones_row")
        nc.vector.memset(ones_row, 1.0)

        eps_t = persist.tile([P, 1], F32, tag="eps")
        nc.vector.memset(eps_t, 1e-5)
        ones_col = persist.tile([P, 1], BF16, tag="ones_col")
        nc.vector.memset(ones_col, 1.0)

        # gamma/beta arranged [p, kt] (e = kt*128 + p)
        gT = persist.tile([P, KT], F32, tag="gT")
        bT = persist.tile([P, KT], F32, tag="bT")
        nc.sync.dma_start(out=gT, in_=ln_g.rearrange("(kt p) -> p kt", p=P))
        nc.sync.dma_start(out=bT, in_=ln_b.rearrange("(kt p) -> p kt", p=P))
        bTh = persist.tile([P, KT], BF16, tag="bTh")
        nc.vector.tensor_copy(out=bTh, in_=bT)

        # w_base -> WbT[p, kt, n] bf16 via DMA transpose (per n-tile)
        WbT = persist.tile([P, KT, DM], BF16, tag="WbT")
        for nt in range(KT):
            wf = ldp.tile([P, E], F32, tag="wload", bufs=3)
            nc.gpsimd.dma_start(out=wf, in_=w_base[nt * P:(nt + 1) * P, :])
            wh = zh_pool.tile([P, E], BF16, tag="zh")
            nc.vector.tensor_copy(out=wh, in_=wf)
            nc.sync.dma_start_transpose(
                out=WbT[:, :, nt * P:(nt + 1) * P], in_=wh)

        # wbsum[n] = sum_e w_base[n, e]  (row vector, via ones matmul)
        wbs_ps = big_ps()
        for grp in range(2):
            for k in range(KT):
                nc.tensor.matmul(wbs_ps[0:1, grp * 512:(grp + 1) * 512],
                                 ones_col, WbT[:, k, grp * 512:(grp + 1) * 512],
                                 start=(k == 0), stop=(k == KT - 1))

        # lora_up scaled, with wbsum appended as row 8 (K=9 fused matmul)
        lu_t = ldp.tile([P, E], F32, tag="wload", bufs=3)
        lu_f = lu_t[0:R, :]
        nc.gpsimd.dma_start(out=lu_f, in_=lu)
        lu8 = persist.tile([R, DM], BF16, tag="lu8")
        nc.scalar.mul(lu8, lu_f, SCALING)
        wbs_sb = persist.tile([1, DM], BF16, tag="wbs_sb")
        nc.vector.tensor_copy(out=wbs_sb, in_=wbs_ps[0:1, :])

        # w_qkv -> two contiguous transposed tensors (qk rows, v rows)
        wqkT = persist.tile([P, KT, P], BF16, tag="wqkT")
        wvT = persist.tile([P, KT, DA], BF16, tag="wvT")
        wq0 = ldp.tile([P, E], F32, tag="wload", bufs=3)
        nc.gpsimd.dma_start(out=wq0, in_=w_qkv[0:P, :])
        wq0h = zh_pool.tile([P, E], BF16, tag="zh")
        nc.vector.tensor_copy(out=wq0h, in_=wq0)
        nc.sync.dma_start_transpose(out=wqkT, in_=wq0h)
        wq1t = ldp.tile([P, E], F32, tag="wload", bufs=3)
        wq1 = wq1t[0:DA, :]
        nc.gpsimd.dma_start(out=wq1, in_=w_qkv[P:3 * DA, :])
        wq1h_t = zh_pool.tile([P, E], BF16, tag="zh")
        wq1h = wq1h_t[0:DA, :]
        nc.vector.tensor_copy(out=wq1h, in_=wq1)
        nc.sync.dma_start_transpose(out=wvT, in_=wq1h)

        # w_attn_out -> waoT [64 d, 1024 e] bf16 via DMA transpose
        waot = ldp.tile([P, E], F32, tag="wload", bufs=3)
        waof = waot[:, 0:KT * DA].rearrange("p (k d) -> p k d", d=DA)
        nc.gpsimd.dma_start(
            out=waof, in_=w_ao.rearrange("(kt p) d -> p kt d", p=P))
        waoh_t = zh_pool.tile([P, E], BF16, tag="zh")
        waoh = waoh_t[:, 0:KT * DA]
        nc.vector.tensor_copy(out=waoh, in_=waof.rearrange("p k d -> p (k d)"))
        waoT = persist.tile([DA, KT, P], BF16, tag="waoT")
        nc.sync.dma_start_transpose(out=waoT, in_=waoh)

        # lora_down [E, R] -> [p, kt, r]; gamma-folded + raw copies
        ld_t = ldp.tile([P, E], F32, tag="wload", bufs=3)
        ld_f = ld_t[:, 0:KT * R].rearrange("p (k r) -> p k r", r=R)
        nc.gpsimd.dma_start(out=ld_f, in_=ld.rearrange("(kt p) r -> p kt r", p=P))
        ld_raw = persist.tile([P, KT, R], BF16, tag="ld_raw")
        nc.vector.tensor_copy(out=ld_raw, in_=ld_f)
        ld_g = persist.tile([P, KT, R], BF16, tag="ld_g")
        for k in range(KT):
            nc.vector.tensor_scalar_mul(out=ld_g[:, k, :], in0=ld_f[:, k, :],
                                        scalar1=gT[:, k:k + 1])
        # bld[r] = sum_e beta[e] * lora_down[e, r]
        bld_ps = mu_ps()
        for k in range(KT):
            nc.tensor.matmul(bld_ps[0:R, 0:1], ld_raw[:, k, :], bTh[:, k:k + 1],
                             start=(k == 0), stop=(k == KT - 1))
        bld = persist.tile([R, 1], F32, tag="bld")
        nc.vector.tensor_copy(out=bld, in_=bld_ps[0:R, 0:1])

        # fold gamma into wqkvT (after bias cols were computed from raw!)
        # bias cols: bqkv_col[a] = sum_e beta[e] wqkv[a, e], a = 0..191
        bqkv0_ps = big_ps()
        for k in range(KT):
            nc.tensor.matmul(bqkv0_ps[0:P, 0:1], wqkT[:, k, :],
                             bTh[:, k:k + 1], start=(k == 0), stop=(k == KT - 1))
        bqkv1_ps = big_ps()
        for k in range(KT):
            nc.tensor.matmul(bqkv1_ps[0:DA, 0:1], wvT[:, k, :],
                             bTh[:, k:k + 1], start=(k == 0), stop=(k == KT - 1))
        bqk_col = persist.tile([P, 1], F32, tag="bqk_col")
        nc.vector.tensor_copy(out=bqk_col, in_=bqkv0_ps[0:P, 0:1])
        bv_col = persist.tile([DA, 1], F32, tag="bv_col")
        nc.vector.tensor_copy(out=bv_col, in_=bqkv1_ps[0:DA, 0:1])
        for k in range(KT):
            nc.vector.tensor_scalar_mul(out=wqkT[:, k, :], in0=wqkT[:, k, :],
                                        scalar1=gT[:, k:k + 1])
            nc.vector.tensor_scalar_mul(out=wvT[:, k, :], in0=wvT[:, k, :],
                                        scalar1=gT[:, k:k + 1])

        # ---------------- persistent activations ----------------
        zT = persist.tile([P, KT, SFULL], BF16, tag="zT")      # 32KB/part
        xcT = persist.tile([P, KT, SOWN], BF16, tag="xcT")     # 16KB/part
        qT = persist.tile([DA, SOWN], BF16, tag="qT")
        kTt = persist.tile([DA, SFULL], BF16, tag="kTt")
        vT = persist.tile([DA, SFULL], BF16, tag="vT")
        v_aug = persist.tile([P, JT, 97], BF16, tag="v_aug")
        v_nat = persist.tile([P, ST, DA], BF16, tag="v_nat")
        aoTn = persist.tile([DA, SOWN], BF16, tag="aoTn")
        aopT = persist.tile([P, KT, SOWN], BF16, tag="aopT")   # 16KB/part
        t8 = persist.tile([R, SOWN], BF16, tag="t8")
        mu_row = persist.tile([1, SOWN], BF16, tag="mu_row")
        mu_all = persist.tile([P, MT], F32, tag="mu_all")

        nc.gpsimd.memset(v_aug, 0.0)
        nc.gpsimd.memset(v_aug[:, 0:ST, DA:DA + 1], 1.0)
        nc.gpsimd.memset(v_aug[:, ST:JT, 96:97], 1.0)

        # ---------------- phase 1: layernorm + DMA transposes ----------------
        for st in range(ST):
            own = st < MT
            src = x_own if own else x_oth
            row0 = st * P if own else (st - MT) * P
            xf = ldp.tile([P, E], F32, tag="xin")
            nc.gpsimd.dma_start(out=xf[:, 0:512], in_=src[row0:row0 + P, 0:512])
            nc.gpsimd.dma_start(out=xf[:, 512:1024],
                                in_=src[row0:row0 + P, 512:1024])

            stats = st_pool.tile([P, 2, 6], F32, tag="bnstats")
            xr = xf.rearrange("p (n f) -> p n f", f=512)
            for sg in range(2):
                nc.vector.bn_stats(out=stats[:, sg, :], in_=xr[:, sg, :])
            mv = st_pool.tile([P, 2], F32, tag="mv")
            nc.vector.bn_aggr(out=mv, in_=stats)

            # rstd = 1/sqrt(var+eps); nmr = -mu*rstd
            rstd = st_pool.tile([P, 1], F32, tag="rstd")
            nc.scalar.activation(out=rstd, in_=mv[:, 1:2], func=AF.Sqrt,
                                 bias=eps_t)
            nc.vector.reciprocal(out=rstd, in_=rstd)
            nmr = st_pool.tile([P, 1], F32, tag="nmr")
            nc.vector.tensor_scalar(out=nmr, in0=mv[:, 0:1], scalar1=rstd,
                                    scalar2=-1.0, op0=MUL, op1=MUL)
            # z = (x - mu) * rstd   (bf16) -> DMA-transpose into zT
            zh = zh_pool.tile([P, E], BF16, tag="zh")
            nc.scalar.activation(out=zh, in_=xf, func=AF.Identity,
                                 scale=rstd, bias=nmr)
            nc.sync.dma_start_transpose(
                out=zT[:, :, st * P:(st + 1) * P], in_=zh)
            if own:
                nc.vector.tensor_copy(out=mu_all[:, st:st + 1], in_=mv[:, 0:1])
                negmu = st_pool.tile([P, 1], F32, tag="negmu")
                nc.vector.tensor_scalar_mul(out=negmu, in0=mv[:, 0:1],
                                            scalar1=-1.0)
                xch = xc_pool.tile([P, E], BF16, tag="xch")
                nc.scalar.activation(out=xch, in_=xf, func=AF.Identity,
                                     bias=negmu)
                nc.sync.dma_start_transpose(
                    out=xcT[:, :, st * P:(st + 1) * P], in_=xch)

        # mu_row[0, m] = mu[m]: psum[0, j] = sum_p mu[p] * I[p, j]
        mu_allh = persist.tile([P, MT], BF16, tag="mu_allh")
        nc.vector.tensor_copy(out=mu_allh, in_=mu_all)
        mur_ps = big_ps()
        for mt in range(MT):
            nc.tensor.matmul(mur_ps[0:1, mt * P:(mt + 1) * P],
                             mu_allh[:, mt:mt + 1], identh,
                             start=True, stop=True)
        nc.vector.tensor_copy(out=mu_row, in_=mur_ps[0:1, :])

        # ---------------- phase 2: qkv projections ----------------
        # q+k rows [0:128] for all 2048 cols (q of other half unused)
        for grp in range(SFULL // 512):
            pqk = big_ps()
            for k in range(KT):
                nc.tensor.matmul(pqk[:, 0:512], wqkT[:, k, :],
                                 zT[:, k, grp * 512:(grp + 1) * 512],
                                 start=(k == 0), stop=(k == KT - 1))
            nc.vector.tensor_scalar_add(
                out=kTt[:, grp * 512:(grp + 1) * 512],
                in0=pqk[DA:P, 0:512], scalar1=bqk_col[DA:P, :])
            if grp < SOWN // 512:
                nc.vector.tensor_scalar_add(
                    out=qT[:, grp * 512:(grp + 1) * 512],
                    in0=pqk[0:DA, 0:512], scalar1=bqk_col[0:DA, :])
        # vT [64 d, 2048 j]
        for grp in range(SFULL // 512):
            pv = big_ps()
            for k in range(KT):
                nc.tensor.matmul(pv[0:DA, 0:512], wvT[:, k, :],
                                 zT[:, k, grp * 512:(grp + 1) * 512],
                                 start=(k == 0), stop=(k == KT - 1))
            nc.vector.tensor_scalar_add(
                out=vT[:, grp * 512:(grp + 1) * 512],
                in0=pv[0:DA, 0:512], scalar1=bv_col)
        # v natural via contiguous-dest DMA transpose, then assemble v_aug:
        # j'-slot jt (head0): [v0 | 0...| 1@64 | 0...]; slot ST+jt (head1):
        # [0... | v1(32:64) | ... | 1@96]
        nc.sync.dma_start_transpose(out=v_nat, in_=vT)
        nc.vector.tensor_copy(out=v_aug[:, 0:ST, 0:HD], in_=v_nat[:, :, 0:HD])
        nc.vector.tensor_copy(out=v_aug[:, ST:JT, HD:DA],
                              in_=v_nat[:, :, HD:DA])

        # ---------------- phase 3: attention ----------------
        # scoresT[j, m] per (head, jt); probs stay transposed; attn@v
        # accumulates over all 32 head-stacked j'-tiles into [66, 512] psums.
        pav = [av_ps() for _ in range(2)]
        for h in range(NH):
            d0 = h * HD
            for jt in range(ST):
                jp = h * ST + jt
                psc = big_ps()
                for mgrp in range(2):
                    nc.tensor.matmul(
                        psc[:, mgrp * 512:(mgrp + 1) * 512],
                        kTt[d0:d0 + HD, jt * P:(jt + 1) * P],
                        qT[d0:d0 + HD, mgrp * 512:(mgrp + 1) * 512],
                        start=True, stop=True)
                pT = pt_pool.tile([P, SOWN], BF16, tag="pT")
                nc.scalar.activation(out=pT, in_=psc, func=AF.Exp,
                                     scale=ATT_SCALE)
                for mgrp in range(2):
                    nc.tensor.matmul(
                        pav[mgrp], v_aug[:, jp, :],
                        pT[:, mgrp * 512:(mgrp + 1) * 512],
                        start=(jp == 0), stop=(jp == JT - 1))
        # normalize: rows 64 / 96 hold per-head denominators; broadcast the
        # reciprocals across partitions with a K=1 ones matmul
        for mgrp in range(2):
            rr0 = st_pool.tile([1, 512], F32, tag="rr0", bufs=1)
            rr1 = st_pool.tile([1, 512], F32, tag="rr1", bufs=1)
            nc.vector.reciprocal(out=rr0, in_=pav[mgrp][DA:DA + 1, :])
            nc.vector.reciprocal(out=rr1, in_=pav[mgrp][96:97, :])
            rrh = st_pool.tile([1, 2, 512], BF16, tag="rrh", bufs=1)
            nc.vector.tensor_copy(out=rrh[:, 0, :], in_=rr0)
            nc.vector.tensor_copy(out=rrh[:, 1, :], in_=rr1)
            rrb_ps = mu_ps()
            nc.tensor.matmul(rrb_ps[0:HD, :], ones_row[0:1, 0:HD],
                             rrh[:, 0, :], start=True, stop=True)
            nc.tensor.matmul(rrb_ps[HD:DA, :], ones_row[0:1, 0:HD],
                             rrh[:, 1, :], start=True, stop=True)
            rrb_sb = st_pool.tile([DA, 512], F32, tag="rrb_sb", bufs=2)
            nc.vector.tensor_copy(out=rrb_sb, in_=rrb_ps[0:DA, :])
            nc.vector.tensor_tensor(
                out=aoTn[:, mgrp * 512:(mgrp + 1) * 512],
                in0=pav[mgrp][0:DA, :], in1=rrb_sb, op=MUL)

        # ---------------- phase 4: attn_out projection ----------------
        for et in range(KT):
            pp = big_ps()
            for mgrp in range(2):
                nc.tensor.matmul(pp[:, mgrp * 512:(mgrp + 1) * 512],
                                 waoT[:, et, :],
                                 aoTn[:, mgrp * 512:(mgrp + 1) * 512],
                                 start=True, stop=True)
            nc.vector.tensor_copy(out=aopT[:, et, :], in_=pp)

        # ---------------- phase 5: lora down ----------------
        # t[r, m] = sum_e (g*ld)[e,r] z[e,m] + ld[e,r] aop[e,m] + bld[r]
        for mgrp in range(2):
            p5 = av_ps()
            for k in range(KT):
                nc.tensor.matmul(p5[0:R, :], ld_g[:, k, :],
                                 zT[:, k, mgrp * 512:(mgrp + 1) * 512],
                                 start=(k == 0), stop=False)
            for k in range(KT):
                nc.tensor.matmul(p5[0:R, :], ld_raw[:, k, :],
                                 aopT[:, k, mgrp * 512:(mgrp + 1) * 512],
                                 start=False, stop=(k == KT - 1))
            nc.vector.tensor_scalar_add(
                out=t8[:, mgrp * 512:(mgrp + 1) * 512],
                in0=p5[0:R, :], scalar1=bld)

        # ---------------- phase 6: base + lora up + mu correction ----------------
        for mt in range(MT):
            p6 = big_ps()
            o_t = o_pool.tile([P, DM], F32, tag="o_t")
            for grp in range(2):
                for k in range(KT):
                    nc.tensor.matmul(p6[:, grp * 512:(grp + 1) * 512],
                                     xcT[:, k, mt * P:(mt + 1) * P],
                                     WbT[:, k, grp * 512:(grp + 1) * 512],
                                     start=(k == 0), stop=False)
                nc.tensor.matmul(p6[:, grp * 512:(grp + 1) * 512],
                                 t8[:, mt * P:(mt + 1) * P],
                                 lu8[:, grp * 512:(grp + 1) * 512],
                                 start=False, stop=False)
                nc.tensor.matmul(p6[:, grp * 512:(grp + 1) * 512],
                                 mu_row[:, mt * P:(mt + 1) * P],
                                 wbs_sb[:, grp * 512:(grp + 1) * 512],
                                 start=False, stop=True)
                nc.scalar.activation(out=o_t[:, grp * 512:(grp + 1) * 512],
                                     in_=p6[:, grp * 512:(grp + 1) * 512],
                                     func=AF.Copy)
                nc.sync.dma_start(
                    out=out_d[mt * P:(mt + 1) * P, grp * 512:(grp + 1) * 512],
                    in_=o_t[:, grp * 512:(grp + 1) * 512])

    nc.compile()
    return nc


_NC_CACHE = None


def _get_nc():
    global _NC_CACHE
    if _NC_CACHE is None:
        _NC_CACHE = build_kernel()
    return _NC_CACHE


def kernel(x, w_base, ln_gamma, ln_beta, lora_down, lora_up, w_qkv, w_attn_out,
           _trace=False):
    x = np.ascontiguousarray(np.asarray(x, dtype=np.float32))
    wk = {
        "w_base": np.ascontiguousarray(np.asarray(w_base, np.float32)),
        "ln_g": np.ascontiguousarray(np.asarray(ln_gamma, np.float32)),
        "ln_b": np.ascontiguousarray(np.asarray(ln_beta, np.float32)),
        "ld": np.ascontiguousarray(np.asarray(lora_down, np.float32)),
        "lu": np.ascontiguousarray(np.asarray(lora_up, np.float32)),
        "w_qkv": np.ascontiguousarray(np.asarray(w_qkv, np.float32)),
        "w_ao": np.ascontiguousarray(np.asarray(w_attn_out, np.float32)),
    }
    nc = _get_nc()
    in_maps = []
    for c in range(NC):
        b, half = divmod(c, 2)
        own = np.ascontiguousarray(x[b, half * SOWN:(half + 1) * SOWN])
        oth = np.ascontiguousarray(x[b, (1 - half) * SOWN:(2 - half) * SOWN])
        in_maps.append({"x_own": own, "x_oth": oth, **wk})
    res = run_bass_kernel_spmd(nc, in_maps, core_ids=list(range(NC)), trace=_trace)
    B, S = x.shape[0], x.shape[1]
    out = np.empty((B, S, DM), np.float32)
    for c in range(NC):
        b, half = divmod(c, 2)
        out[b, half * SOWN:(half + 1) * SOWN] = res.results[c]["out"]
    if _trace:
        kernel.last_exec_time_ns = res.exec_time_ns
        kernel.last_results = res
    return out


# revision 12
# speedup vs baseline: 1.0906x; 1.0906x over previous
"""Trainium2 Bass kernel for the LoRA-BC block (nn_LoRABCBlock).

Computation (per reference):
    base = x @ w_base.T
    h = layernorm(x) * gamma + beta
    qkv = h @ w_qkv.T ; attention (2 heads, head_dim 32) over full sequence
    attn_out = attn_output @ w_attn_out.T
    delta = ((h + attn_out) @ lora_down) @ lora_up
    out = base + (1/8) * delta

Sharding: data-parallel over (batch, seq-half) -> 8 cores. Each core owns
1024 query rows of one batch element, computes k/v over that batch's full
2048-row sequence, and produces its 1024 output rows. Weights replicated.
No collectives.

Structure (v3):
  - All activation/weight transposes go through the DMA XBAR
    (dma_start_transpose) with contiguous or verified-stride destinations;
    activations are split into per-consumer tensors (zT in 512-col groups,
    xcT per m-tile) so readers start as soon as their slice lands.
  - gamma/beta folded into w_qkv / lora_down + bias columns; the base
    matmul runs on centered xc=(x-mu) with the mu (x) colsum(w_base) rank-1
    term fused as a K=1 matmul and the rank-8 LoRA update as a K=8 matmul.
  - Attention computes transposed scores (scoresT[j,m]); probabilities come
    out in the layout attn@v needs. Per head, 8-jt chunks of scores+exp are
    batched, then attn@v runs 16 back-to-back PE matmuls per chunk into
    per-head [33, 512] accumulators whose row 32 is the softmax denominator
    (an appended ones-column of V). Denominator reciprocals are broadcast
    across partitions with K=1 ones matmuls.
"""

import sys

sys.path.insert(0, "/opt/trn_rl_repo")

from contextlib import ExitStack

import numpy as np

import concourse.bass as bass
import concourse.tile as tile
from concourse import bacc, mybir
from concourse.bass_utils import run_bass_kernel_spmd
from concourse.masks import make_identity

F32 = mybir.dt.float32
BF16 = mybir.dt.bfloat16
AF = mybir.ActivationFunctionType
MUL = mybir.AluOpType.mult

E = 1024          # embed dim
DM = 1024         # d_model
R = 8             # lora rank
SCALING = 1.0 / R
DA = 64           # attn dim
NH = 2            # heads
HD = DA // NH     # head dim = 32
SOWN = 1024       # rows owned per core
SFULL = 2048      # rows per batch element
NC = 8            # cores
P = 128
KT = E // P       # 8 k-tiles
MT = SOWN // P    # 8 own m-tiles
ST = SFULL // P   # 16 sequence tiles
NG = SFULL // 512  # 4 zT column groups
ATT_SCALE = float(HD) ** -0.5


def build_kernel():
    nc = bacc.Bacc("TRN2", target_bir_lowering=False, debug=False, num_devices=NC)

    x_own = nc.dram_tensor("x_own", [SOWN, E], F32, kind="ExternalInput").ap()
    x_oth = nc.dram_tensor("x_oth", [SOWN, E], F32, kind="ExternalInput").ap()
    w_base = nc.dram_tensor("w_base", [DM, E], F32, kind="ExternalInput").ap()
    ln_g = nc.dram_tensor("ln_g", [E], F32, kind="ExternalInput").ap()
    ln_b = nc.dram_tensor("ln_b", [E], F32, kind="ExternalInput").ap()
    ld = nc.dram_tensor("ld", [E, R], F32, kind="ExternalInput").ap()
    lu = nc.dram_tensor("lu", [R, DM], F32, kind="ExternalInput").ap()
    w_qkv = nc.dram_tensor("w_qkv", [3 * DA, E], F32, kind="ExternalInput").ap()
    w_ao = nc.dram_tensor("w_ao", [E, DA], F32, kind="ExternalInput").ap()
    out_d = nc.dram_tensor("out", [SOWN, DM], F32, kind="ExternalOutput").ap()

    with tile.TileContext(nc) as tc, ExitStack() as ctx:
        persist = ctx.enter_context(tc.tile_pool(name="persist", bufs=1))
        ldp = ctx.enter_context(tc.tile_pool(name="loads", bufs=3))
        zh_pool = ctx.enter_context(tc.tile_pool(name="zh", bufs=3))
        xc_pool = ctx.enter_context(tc.tile_pool(name="xc", bufs=2))
        st_pool = ctx.enter_context(tc.tile_pool(name="stats", bufs=4))
        pt_pool = ctx.enter_context(tc.tile_pool(name="pt", bufs=2))
        o_pool = ctx.enter_context(tc.tile_pool(name="outs", bufs=2))
        ps = ctx.enter_context(tc.tile_pool(name="ps", bufs=1, space="PSUM"))

        _n = [0]

        def ps_tile(shape, tag, bufs):
            _n[0] += 1
            return ps.tile(shape, F32, tag=tag, bufs=bufs,
                           name=f"ps_{tag}_{_n[0]}")

        def big_ps():
            # [128, 1024] fp32 = 2 PSUM banks; matmuls target 512-wide halves
            return ps_tile([P, 1024], "big", 2)

        def av_ps():
            return ps_tile([HD + 1, 512], "av", 4)

        # ---------------- phase 0: constants + weights ----------------
        identh = persist.tile([P, P], BF16, tag="identh")
        make_identity(nc, identh)

        eps_t = persist.tile([P, 1], F32, tag="eps")
        nc.vector.memset(eps_t, 1e-5)
        ones_col = persist.tile([P, 1], BF16, tag="ones_col")
        nc.vector.memset(ones_col, 1.0)
        ones_row = persist.tile([1, DA], BF16, tag="ones_row")
        nc.vector.memset(ones_row, 1.0)

        # gamma/beta arranged [p, kt] (e = kt*128 + p)
        gT = persist.tile([P, KT], F32, tag="gT")
        bT = persist.tile([P, KT], F32, tag="bT")
        nc.sync.dma_start(out=gT, in_=ln_g.rearrange("(kt p) -> p kt", p=P))
        nc.sync.dma_start(out=bT, in_=ln_b.rearrange("(kt p) -> p kt", p=P))
        bTh = persist.tile([P, KT], BF16, tag="bTh")
        nc.vector.tensor_copy(out=bTh, in_=bT)

        # w_base -> WbT[p, kt, n] bf16 via DMA transpose (per n-tile)
        WbT = persist.tile([P, KT, DM], BF16, tag="WbT")
        for nt in range(KT):
            wf = ldp.tile([P, E], F32, tag="wload", bufs=3)
            nc.gpsimd.dma_start(out=wf, in_=w_base[nt * P:(nt + 1) * P, :])
            wh = zh_pool.tile([P, E], BF16, tag="zh")
            nc.vector.tensor_copy(out=wh, in_=wf)
            nc.scalar.dma_start_transpose(
                out=WbT[:, :, nt * P:(nt + 1) * P], in_=wh)

        # wbsum[n] = sum_e w_base[n, e]  (row vector, via ones matmul)
        wbs_ps = big_ps()
        for grp in range(2):
            for k in range(KT):
                nc.tensor.matmul(wbs_ps[0:1, grp * 512:(grp + 1) * 512],
                                 ones_col, WbT[:, k, grp * 512:(grp + 1) * 512],
                                 start=(k == 0), stop=(k == KT - 1))
        wbs_sb = persist.tile([1, DM], BF16, tag="wbs_sb")
        nc.vector.tensor_copy(out=wbs_sb, in_=wbs_ps[0:1, :])

        # lora_up scaled
        lu_t = ldp.tile([P, E], F32, tag="wload", bufs=3)
        lu_f = lu_t[0:R, :]
        nc.gpsimd.dma_start(out=lu_f, in_=lu)
        lu8 = persist.tile([R, DM], BF16, tag="lu8")
        nc.scalar.mul(lu8, lu_f, SCALING)

        # w_qkv -> two contiguous transposed tensors (qk rows, v rows)
        wqkT = persist.tile([P, KT, P], BF16, tag="wqkT")
        wvT = persist.tile([P, KT, DA], BF16, tag="wvT")
        wq0 = ldp.tile([P, E], F32, tag="wload", bufs=3)
        nc.gpsimd.dma_start(out=wq0, in_=w_qkv[0:P, :])
        wq0h = zh_pool.tile([P, E], BF16, tag="zh")
        nc.vector.tensor_copy(out=wq0h, in_=wq0)
        nc.scalar.dma_start_transpose(out=wqkT, in_=wq0h)
        wq1t = ldp.tile([P, E], F32, tag="wload", bufs=3)
        wq1 = wq1t[0:DA, :]
        nc.gpsimd.dma_start(out=wq1, in_=w_qkv[P:3 * DA, :])
        wq1h_t = zh_pool.tile([P, E], BF16, tag="zh")
        wq1h = wq1h_t[0:DA, :]
        nc.vector.tensor_copy(out=wq1h, in_=wq1)
        nc.scalar.dma_start_transpose(out=wvT, in_=wq1h)

        # w_attn_out -> waoT [64 d, kt, 128] bf16 via DMA transpose
        waot = ldp.tile([P, E], F32, tag="wload", bufs=3)
        waof = waot[:, 0:KT * DA].rearrange("p (k d) -> p k d", d=DA)
        nc.gpsimd.dma_start(
            out=waof, in_=w_ao.rearrange("(kt p) d -> p kt d", p=P))
        waoh_t = zh_pool.tile([P, E], BF16, tag="zh")
        waoh = waoh_t[:, 0:KT * DA]
        nc.vector.tensor_copy(out=waoh, in_=waof.rearrange("p k d -> p (k d)"))
        waoT = persist.tile([DA, KT, P], BF16, tag="waoT")
        nc.scalar.dma_start_transpose(out=waoT, in_=waoh)

        # lora_down [E, R] -> [p, kt, r]; gamma-folded + raw copies
        ld_t = ldp.tile([P, E], F32, tag="wload", bufs=3)
        ld_f = ld_t[:, 0:KT * R].rearrange("p (k r) -> p k r", r=R)
        nc.gpsimd.dma_start(out=ld_f, in_=ld.rearrange("(kt p) r -> p kt r", p=P))
        ld_raw = persist.tile([P, KT, R], BF16, tag="ld_raw")
        nc.vector.tensor_copy(out=ld_raw, in_=ld_f)
        ld_g = persist.tile([P, KT, R], BF16, tag="ld_g")
        for k in range(KT):
            nc.vector.tensor_scalar_mul(out=ld_g[:, k, :], in0=ld_f[:, k, :],
                                        scalar1=gT[:, k:k + 1])
        # bld[r] = sum_e beta[e] * lora_down[e, r]
        bld_ps = av_ps()
        for k in range(KT):
            nc.tensor.matmul(bld_ps[0:R, 0:1], ld_raw[:, k, :], bTh[:, k:k + 1],
                             start=(k == 0), stop=(k == KT - 1))
        bld = persist.tile([R, 1], F32, tag="bld")
        nc.vector.tensor_copy(out=bld, in_=bld_ps[0:R, 0:1])

        # qkv bias columns from beta (raw weights, pre-gamma-fold)
        bqkv0_ps = big_ps()
        for k in range(KT):
            nc.tensor.matmul(bqkv0_ps[0:P, 0:1], wqkT[:, k, :],
                             bTh[:, k:k + 1], start=(k == 0), stop=(k == KT - 1))
        bqkv1_ps = big_ps()
        for k in range(KT):
            nc.tensor.matmul(bqkv1_ps[0:DA, 0:1], wvT[:, k, :],
                             bTh[:, k:k + 1], start=(k == 0), stop=(k == KT - 1))
        bqk_col = persist.tile([P, 1], F32, tag="bqk_col")
        nc.vector.tensor_copy(out=bqk_col, in_=bqkv0_ps[0:P, 0:1])
        bv_col = persist.tile([DA, 1], F32, tag="bv_col")
        nc.vector.tensor_copy(out=bv_col, in_=bqkv1_ps[0:DA, 0:1])
        for k in range(KT):
            nc.vector.tensor_scalar_mul(out=wqkT[:, k, :], in0=wqkT[:, k, :],
                                        scalar1=gT[:, k:k + 1])
            nc.vector.tensor_scalar_mul(out=wvT[:, k, :], in0=wvT[:, k, :],
                                        scalar1=gT[:, k:k + 1])

        # ---------------- persistent activations ----------------
        # zT split into 512-col groups, xcT per m-tile: consumers start as
        # soon as their group's transposes land.
        zTg = [persist.tile([P, KT, 512], BF16, tag=f"zTg{g}", name=f"zTg{g}")
               for g in range(NG)]
        xcm = [persist.tile([P, KT, P], BF16, tag=f"xcm{m}", name=f"xcm{m}")
               for m in range(MT)]
        qT = persist.tile([DA, SOWN], BF16, tag="qT")
        kTt = persist.tile([DA, SFULL], BF16, tag="kTt")
        vT = persist.tile([DA, SFULL], BF16, tag="vT")
        v_nat = persist.tile([P, ST, DA], BF16, tag="v_nat")
        v_aug = persist.tile([P, NH, ST, HD + 1], BF16, tag="v_aug")
        aoTn = persist.tile([DA, SOWN], BF16, tag="aoTn")
        aopT = persist.tile([P, KT, SOWN], BF16, tag="aopT")
        t8 = persist.tile([R, SOWN], BF16, tag="t8")
        mu_row = persist.tile([1, SOWN], BF16, tag="mu_row")
        mu_all = persist.tile([P, MT], F32, tag="mu_all")

        nc.gpsimd.memset(v_aug[:, :, :, HD:HD + 1], 1.0)

        # ---------------- phase 1: layernorm + DMA transposes ----------------
        for st in range(ST):
            own = st < MT
            src = x_own if own else x_oth
            row0 = st * P if own else (st - MT) * P
            xf = ldp.tile([P, E], F32, tag="xin")
            nc.gpsimd.dma_start(out=xf, in_=src[row0:row0 + P, :])

            stats = st_pool.tile([P, 2, 6], F32, tag="bnstats")
            xr = xf.rearrange("p (n f) -> p n f", f=512)
            for sg in range(2):
                nc.vector.bn_stats(out=stats[:, sg, :], in_=xr[:, sg, :])
            mv = st_pool.tile([P, 2], F32, tag="mv")
            nc.vector.bn_aggr(out=mv, in_=stats)

            # rstd = 1/sqrt(var+eps); nmr = -mu*rstd
            rstd = st_pool.tile([P, 1], F32, tag="rstd")
            nc.scalar.activation(out=rstd, in_=mv[:, 1:2], func=AF.Sqrt,
                                 bias=eps_t)
            nc.vector.reciprocal(out=rstd, in_=rstd)
            nmr = st_pool.tile([P, 1], F32, tag="nmr")
            nc.vector.tensor_scalar(out=nmr, in0=mv[:, 0:1], scalar1=rstd,
                                    scalar2=-1.0, op0=MUL, op1=MUL)
            # z = (x - mu) * rstd   (bf16) -> DMA-transpose into zT group
            zh = zh_pool.tile([P, E], BF16, tag="zh")
            nc.scalar.activation(out=zh, in_=xf, func=AF.Identity,
                                 scale=rstd, bias=nmr)
            g, sub = divmod(st, 4)
            nc.scalar.dma_start_transpose(
                out=zTg[g][:, :, sub * P:(sub + 1) * P], in_=zh)
            if own:
                nc.vector.tensor_copy(out=mu_all[:, st:st + 1], in_=mv[:, 0:1])
                negmu = st_pool.tile([P, 1], F32, tag="negmu")
                nc.vector.tensor_scalar_mul(out=negmu, in0=mv[:, 0:1],
                                            scalar1=-1.0)
                xch = xc_pool.tile([P, E], BF16, tag="xch")
                nc.scalar.activation(out=xch, in_=xf, func=AF.Identity,
                                     bias=negmu)
                nc.scalar.dma_start_transpose(out=xcm[st], in_=xch)

        # mu_row[0, m] = mu[m]: psum[0, j] = sum_p mu[p] * I[p, j]
        mu_allh = persist.tile([P, MT], BF16, tag="mu_allh")
        nc.vector.tensor_copy(out=mu_allh, in_=mu_all)
        mur_ps = big_ps()
        for mt in range(MT):
            nc.tensor.matmul(mur_ps[0:1, mt * P:(mt + 1) * P],
                             mu_allh[:, mt:mt + 1], identh,
                             start=True, stop=True)
        nc.vector.tensor_copy(out=mu_row, in_=mur_ps[0:1, :])

        # ---------------- phase 2: qkv projections ----------------
        # q+k rows [0:128] for all 2048 cols (q of other half unused)
        for g in range(NG):
            pqk = big_ps()
            for k in range(KT):
                nc.tensor.matmul(pqk[:, 0:512], wqkT[:, k, :], zTg[g][:, k, :],
                                 start=(k == 0), stop=(k == KT - 1))
            nc.vector.tensor_scalar_add(
                out=kTt[:, g * 512:(g + 1) * 512],
                in0=pqk[DA:P, 0:512], scalar1=bqk_col[DA:P, :])
            if g < 2:
                nc.vector.tensor_scalar_add(
                    out=qT[:, g * 512:(g + 1) * 512],
                    in0=pqk[0:DA, 0:512], scalar1=bqk_col[0:DA, :])
        # vT [64 d, 2048 j]
        for g in range(NG):
            pv = big_ps()
            for k in range(KT):
                nc.tensor.matmul(pv[0:DA, 0:512], wvT[:, k, :], zTg[g][:, k, :],
                                 start=(k == 0), stop=(k == KT - 1))
            nc.vector.tensor_scalar_add(
                out=vT[:, g * 512:(g + 1) * 512],
                in0=pv[0:DA, 0:512], scalar1=bv_col)
        # v natural via contiguous-dest DMA transpose, then per-head v_aug
        # (col 32 of each head slot is the pre-set ones column)
        nc.scalar.dma_start_transpose(out=v_nat, in_=vT)
        nc.vector.tensor_copy(out=v_aug[:, 0, :, 0:HD], in_=v_nat[:, :, 0:HD])
        nc.vector.tensor_copy(out=v_aug[:, 1, :, 0:HD], in_=v_nat[:, :, HD:DA])

        # ---------------- phase 3: attention ----------------
        # Per head: two 8-jt chunks. Each chunk: 16 scoresT matmuls + 8 exps
        # into an SBUF chunk buffer, then 16 back-to-back attn@v matmuls
        # accumulating into per-head [33, 512] psums (row 32 = denominator).
        CH = 8  # jt per chunk
        for h in range(NH):
            d0 = h * HD
            pav = [av_ps() for _ in range(2)]
            for chunk in range(ST // CH):
                pTc = pt_pool.tile([P, CH, SOWN], BF16, tag="pT")
                for j8 in range(CH):
                    jt = chunk * CH + j8
                    psc = big_ps()
                    for mgrp in range(2):
                        nc.tensor.matmul(
                            psc[:, mgrp * 512:(mgrp + 1) * 512],
                            kTt[d0:d0 + HD, jt * P:(jt + 1) * P],
                            qT[d0:d0 + HD, mgrp * 512:(mgrp + 1) * 512],
                            start=True, stop=True)
                    nc.scalar.activation(out=pTc[:, j8, :], in_=psc,
                                         func=AF.Exp, scale=ATT_SCALE)
                for j8 in range(CH):
                    jt = chunk * CH + j8
                    for mgrp in range(2):
                        nc.tensor.matmul(
                            pav[mgrp], v_aug[:, h, jt, :],
                            pTc[:, j8, mgrp * 512:(mgrp + 1) * 512],
                            start=(jt == 0), stop=(jt == ST - 1))
            # normalize head h: broadcast 1/denominator via K=1 ones matmul
            for mgrp in range(2):
                rr = st_pool.tile([1, 512], F32, tag="rr", bufs=2)
                nc.vector.reciprocal(out=rr, in_=pav[mgrp][HD:HD + 1, :])
                rrh = st_pool.tile([1, 512], BF16, tag="rrh", bufs=2)
                nc.vector.tensor_copy(out=rrh, in_=rr)
                rrb_ps = av_ps()
                nc.tensor.matmul(rrb_ps[0:HD, :], ones_row[0:1, 0:HD],
                                 rrh, start=True, stop=True)
                rrb_sb = st_pool.tile([HD, 512], F32, tag="rrb_sb", bufs=2)
                nc.vector.tensor_copy(out=rrb_sb, in_=rrb_ps[0:HD, :])
                nc.vector.tensor_tensor(
                    out=aoTn[d0:d0 + HD, mgrp * 512:(mgrp + 1) * 512],
                    in0=pav[mgrp][0:HD, :], in1=rrb_sb, op=MUL)

        # ---------------- phase 4: attn_out projection ----------------
        for et in range(KT):
            pp = big_ps()
            for mgrp in range(2):
                nc.tensor.matmul(pp[:, mgrp * 512:(mgrp + 1) * 512],
                                 waoT[:, et, :],
                                 aoTn[:, mgrp * 512:(mgrp + 1) * 512],
                                 start=True, stop=True)
            nc.vector.tensor_copy(out=aopT[:, et, :], in_=pp)

        # ---------------- phase 5: lora down ----------------
        # t[r, m] = sum_e (g*ld)[e,r] z[e,m] + ld[e,r] aop[e,m] + bld[r]
        for mgrp in range(2):
            p5 = av_ps()
            for k in range(KT):
                nc.tensor.matmul(p5[0:R, :], ld_g[:, k, :], zTg[mgrp][:, k, :],
                                 start=(k == 0), stop=False)
            for k in range(KT):
                nc.tensor.matmul(p5[0:R, :], ld_raw[:, k, :],
                                 aopT[:, k, mgrp * 512:(mgrp + 1) * 512],
                                 start=False, stop=(k == KT - 1))
            nc.vector.tensor_scalar_add(
                out=t8[:, mgrp * 512:(mgrp + 1) * 512],
                in0=p5[0:R, :], scalar1=bld)

        # ---------------- phase 6: base + lora up + mu correction ----------------
        for mt in range(MT):
            p6 = big_ps()
            o_t = o_pool.tile([P, DM], F32, tag="o_t")
            for grp in range(2):
                for k in range(KT):
                    nc.tensor.matmul(p6[:, grp * 512:(grp + 1) * 512],
                                     xcm[mt][:, k, :],
                                     WbT[:, k, grp * 512:(grp + 1) * 512],
                                     start=(k == 0), stop=False)
                nc.tensor.matmul(p6[:, grp * 512:(grp + 1) * 512],
                                 t8[:, mt * P:(mt + 1) * P],
                                 lu8[:, grp * 512:(grp + 1) * 512],
                                 start=False, stop=False)
                nc.tensor.matmul(p6[:, grp * 512:(grp + 1) * 512],
                                 mu_row[:, mt * P:(mt + 1) * P],
                                 wbs_sb[:, grp * 512:(grp + 1) * 512],
                                 start=False, stop=True)
                nc.scalar.activation(out=o_t[:, grp * 512:(grp + 1) * 512],
                                     in_=p6[:, grp * 512:(grp + 1) * 512],
                                     func=AF.Copy)
                nc.sync.dma_start(
                    out=out_d[mt * P:(mt + 1) * P, grp * 512:(grp + 1) * 512],
                    in_=o_t[:, grp * 512:(grp + 1) * 512])

    nc.compile()
    return nc


_NC_CACHE = None


def _get_nc():
    global _NC_CACHE
    if _NC_CACHE is None:
        _NC_CACHE = build_kernel()
    return _NC_CACHE


def kernel(x, w_base, ln_gamma, ln_beta, lora_down, lora_up, w_qkv, w_attn_out,
           _trace=False):
    x = np.ascontiguousarray(np.asarray(x, dtype=np.float32))
    wk = {
        "w_base": np.ascontiguousarray(np.asarray(w_base, np.float32)),
        "ln_g": np.ascontiguousarray(np.asarray(ln_gamma, np.float32)),
        "ln_b": np.ascontiguousarray(np.asarray(ln_beta, np.float32)),
        "ld": np.ascontiguousarray(np.asarray(lora_down, np.float32)),
        "lu": np.ascontiguousarray(np.asarray(lora_up, np.float32)),
        "w_qkv": np.ascontiguousarray(np.asarray(w_qkv, np.float32)),
        "w_ao": np.ascontiguousarray(np.asarray(w_attn_out, np.float32)),
    }
    nc = _get_nc()
    in_maps = []
    for c in range(NC):
        b, half = divmod(c, 2)
        own = np.ascontiguousarray(x[b, half * SOWN:(half + 1) * SOWN])
        oth = np.ascontiguousarray(x[b, (1 - half) * SOWN:(2 - half) * SOWN])
        in_maps.append({"x_own": own, "x_oth": oth, **wk})
    res = run_bass_kernel_spmd(nc, in_maps, core_ids=list(range(NC)), trace=_trace)
    B, S = x.shape[0], x.shape[1]
    out = np.empty((B, S, DM), np.float32)
    for c in range(NC):
        b, half = divmod(c, 2)
        out[b, half * SOWN:(half + 1) * SOWN] = res.results[c]["out"]
    if _trace:
        kernel.last_exec_time_ns = res.exec_time_ns
        kernel.last_results = res
    return out


# revision 13
# speedup vs baseline: 1.3609x; 1.2479x over previous
"""Trainium2 Bass kernel for the LoRA-BC block (nn_LoRABCBlock).

Computation (per reference):
    base = x @ w_base.T
    h = layernorm(x) * gamma + beta
    qkv = h @ w_qkv.T ; attention (2 heads, head_dim 32) over full sequence
    attn_out = attn_output @ w_attn_out.T
    delta = ((h + attn_out) @ lora_down) @ lora_up
    out = base + (1/8) * delta

Sharding: data-parallel over (batch, seq-half) -> 8 cores. Each core owns
1024 query rows of one batch element, computes k/v over that batch's full
2048-row sequence, and produces its 1024 output rows. Weights replicated.
No collectives.

Structure (v4):
  - Weight transposes ride the DMA XBAR (slow ~7us/tile but off the
    critical path, split across the SP and ACT hwdge queues); activation
    transposes run on the PE (8 per z-tile batched into one [128,1024]
    bf16 psum, one wide DVE evict into the 512-col zT group tensors).
  - gamma/beta folded into w_qkv / lora_down + bias columns. The base
    matmul runs on zT directly: out = std[m]*(z@Wb^T) + mu x wbsum + delta,
    evicted with a per-partition std scale; the rank-8 LoRA + rank-1 mu
    term accumulate in a second psum added at eviction.
  - Attention computes transposed scores (scoresT[j,m]); probabilities come
    out in the layout attn@v needs. Per head, 8-jt chunks of scores+exp are
    batched, then attn@v runs back-to-back PE matmuls into per-head
    [33, 512] accumulators whose row 32 is the softmax denominator (an
    appended ones-column of V). Denominator reciprocals are broadcast
    across partitions with K=1 ones matmuls.
"""

import sys

sys.path.insert(0, "/opt/trn_rl_repo")

from contextlib import ExitStack

import numpy as np

import concourse.bass as bass
import concourse.tile as tile
from concourse import bacc, mybir
from concourse.bass_utils import run_bass_kernel_spmd
from concourse.masks import make_identity

F32 = mybir.dt.float32
BF16 = mybir.dt.bfloat16
AF = mybir.ActivationFunctionType
MUL = mybir.AluOpType.mult

E = 1024          # embed dim
DM = 1024         # d_model
R = 8             # lora rank
SCALING = 1.0 / R
DA = 64           # attn dim
NH = 2            # heads
HD = DA // NH     # head dim = 32
SOWN = 1024       # rows owned per core
SFULL = 2048      # rows per batch element
NC = 8            # cores
P = 128
KT = E // P       # 8 k-tiles
MT = SOWN // P    # 8 own m-tiles
ST = SFULL // P   # 16 sequence tiles
NG = SFULL // 512  # 4 zT column groups
ATT_SCALE = float(HD) ** -0.5


def build_kernel():
    nc = bacc.Bacc("TRN2", target_bir_lowering=False, debug=False, num_devices=NC)

    x_own = nc.dram_tensor("x_own", [SOWN, E], F32, kind="ExternalInput").ap()
    x_oth = nc.dram_tensor("x_oth", [SOWN, E], F32, kind="ExternalInput").ap()
    w_base = nc.dram_tensor("w_base", [DM, E], F32, kind="ExternalInput").ap()
    ln_g = nc.dram_tensor("ln_g", [E], F32, kind="ExternalInput").ap()
    ln_b = nc.dram_tensor("ln_b", [E], F32, kind="ExternalInput").ap()
    ld = nc.dram_tensor("ld", [E, R], F32, kind="ExternalInput").ap()
    lu = nc.dram_tensor("lu", [R, DM], F32, kind="ExternalInput").ap()
    w_qkv = nc.dram_tensor("w_qkv", [3 * DA, E], F32, kind="ExternalInput").ap()
    w_ao = nc.dram_tensor("w_ao", [E, DA], F32, kind="ExternalInput").ap()
    out_d = nc.dram_tensor("out", [SOWN, DM], F32, kind="ExternalOutput").ap()

    with tile.TileContext(nc) as tc, ExitStack() as ctx:
        persist = ctx.enter_context(tc.tile_pool(name="persist", bufs=1))
        ldp = ctx.enter_context(tc.tile_pool(name="loads", bufs=3))
        zh_pool = ctx.enter_context(tc.tile_pool(name="zh", bufs=3))
        st_pool = ctx.enter_context(tc.tile_pool(name="stats", bufs=4))
        pt_pool = ctx.enter_context(tc.tile_pool(name="pt", bufs=2))
        o_pool = ctx.enter_context(tc.tile_pool(name="outs", bufs=2))
        ps = ctx.enter_context(tc.tile_pool(name="ps", bufs=1, space="PSUM"))

        _n = [0]

        def ps_tile(shape, dtype, tag, bufs):
            _n[0] += 1
            return ps.tile(shape, dtype, tag=tag, bufs=bufs,
                           name=f"ps_{tag}_{_n[0]}")

        def mm_ps():
            return ps_tile([P, 512], F32, "mm", 3)

        def av_ps():
            return ps_tile([HD + 1, 512], F32, "av", 3)

        def tp_ps():
            return ps_tile([P, E], BF16, "tp", 2)

        # ---------------- phase 0: constants + weights ----------------
        identh = persist.tile([P, P], BF16, tag="identh")
        make_identity(nc, identh)

        eps_t = persist.tile([P, 1], F32, tag="eps")
        nc.vector.memset(eps_t, 1e-5)
        ones_col = persist.tile([P, 1], BF16, tag="ones_col")
        nc.vector.memset(ones_col, 1.0)
        ones_row = persist.tile([1, DA], BF16, tag="ones_row")
        nc.vector.memset(ones_row, 1.0)

        # gamma/beta arranged [p, kt] (e = kt*128 + p)
        gT = persist.tile([P, KT], F32, tag="gT")
        bT = persist.tile([P, KT], F32, tag="bT")
        nc.sync.dma_start(out=gT, in_=ln_g.rearrange("(kt p) -> p kt", p=P))
        nc.sync.dma_start(out=bT, in_=ln_b.rearrange("(kt p) -> p kt", p=P))
        bTh = persist.tile([P, KT], BF16, tag="bTh")
        nc.vector.tensor_copy(out=bTh, in_=bT)

        # w_qkv -> two contiguous transposed tensors (ACT hwdge queue,
        # ahead of the bulky w_base tiles which ride the SP queue)
        wqkT = persist.tile([P, KT, P], BF16, tag="wqkT")
        wvT = persist.tile([P, KT, DA], BF16, tag="wvT")
        wq0 = ldp.tile([P, E], F32, tag="wload", bufs=3)
        nc.gpsimd.dma_start(out=wq0, in_=w_qkv[0:P, :])
        wq0h = zh_pool.tile([P, E], BF16, tag="zh")
        nc.vector.tensor_copy(out=wq0h, in_=wq0)
        nc.scalar.dma_start_transpose(out=wqkT, in_=wq0h)
        wq1t = ldp.tile([P, E], F32, tag="wload", bufs=3)
        wq1 = wq1t[0:DA, :]
        nc.gpsimd.dma_start(out=wq1, in_=w_qkv[P:3 * DA, :])
        wq1h_t = zh_pool.tile([P, E], BF16, tag="zh")
        wq1h = wq1h_t[0:DA, :]
        nc.vector.tensor_copy(out=wq1h, in_=wq1)
        nc.scalar.dma_start_transpose(out=wvT, in_=wq1h)

        # w_attn_out -> waoT [64 d, kt, 128] (ACT queue)
        waot = ldp.tile([P, E], F32, tag="wload", bufs=3)
        waof = waot[:, 0:KT * DA].rearrange("p (k d) -> p k d", d=DA)
        nc.gpsimd.dma_start(
            out=waof, in_=w_ao.rearrange("(kt p) d -> p kt d", p=P))
        waoh_t = zh_pool.tile([P, E], BF16, tag="zh")
        waoh = waoh_t[:, 0:KT * DA]
        nc.vector.tensor_copy(out=waoh, in_=waof.rearrange("p k d -> p (k d)"))
        waoT = persist.tile([DA, KT, P], BF16, tag="waoT")
        nc.scalar.dma_start_transpose(out=waoT, in_=waoh)

        # w_base -> WbT[p, kt, n] via DMA transpose on the SP queue (slow,
        # ~7us/tile, but only needed by wbsum/phase 6)
        WbT = persist.tile([P, KT, DM], BF16, tag="WbT")
        for nt in range(KT):
            wf = ldp.tile([P, E], F32, tag="wload", bufs=3)
            nc.gpsimd.dma_start(out=wf, in_=w_base[nt * P:(nt + 1) * P, :])
            wh = zh_pool.tile([P, E], BF16, tag="zh")
            nc.vector.tensor_copy(out=wh, in_=wf)
            nc.sync.dma_start_transpose(
                out=WbT[:, :, nt * P:(nt + 1) * P], in_=wh)

        # wbsum[n] = sum_e w_base[n, e]  (row vector, via ones matmul)
        wbs_sb = persist.tile([1, DM], BF16, tag="wbs_sb")
        for grp in range(2):
            wbs_ps = mm_ps()
            for k in range(KT):
                nc.tensor.matmul(wbs_ps[0:1, :], ones_col,
                                 WbT[:, k, grp * 512:(grp + 1) * 512],
                                 start=(k == 0), stop=(k == KT - 1))
            nc.vector.tensor_copy(out=wbs_sb[:, grp * 512:(grp + 1) * 512],
                                  in_=wbs_ps[0:1, :])

        # lora_up scaled
        lu_t = ldp.tile([P, E], F32, tag="wload", bufs=3)
        lu_f = lu_t[0:R, :]
        nc.gpsimd.dma_start(out=lu_f, in_=lu)
        lu8 = persist.tile([R, DM], BF16, tag="lu8")
        nc.scalar.mul(lu8, lu_f, SCALING)

        # lora_down [E, R] -> [p, kt, r]; gamma-folded + raw copies
        ld_t = ldp.tile([P, E], F32, tag="wload", bufs=3)
        ld_f = ld_t[:, 0:KT * R].rearrange("p (k r) -> p k r", r=R)
        nc.gpsimd.dma_start(out=ld_f, in_=ld.rearrange("(kt p) r -> p kt r", p=P))
        ld_raw = persist.tile([P, KT, R], BF16, tag="ld_raw")
        nc.vector.tensor_copy(out=ld_raw, in_=ld_f)
        ld_g = persist.tile([P, KT, R], BF16, tag="ld_g")
        for k in range(KT):
            nc.vector.tensor_scalar_mul(out=ld_g[:, k, :], in0=ld_f[:, k, :],
                                        scalar1=gT[:, k:k + 1])
        # bld[r] = sum_e beta[e] * lora_down[e, r]
        bld_ps = av_ps()
        for k in range(KT):
            nc.tensor.matmul(bld_ps[0:R, 0:1], ld_raw[:, k, :], bTh[:, k:k + 1],
                             start=(k == 0), stop=(k == KT - 1))
        bld = persist.tile([R, 1], F32, tag="bld")
        nc.vector.tensor_copy(out=bld, in_=bld_ps[0:R, 0:1])

        # qkv bias columns from beta (raw weights, pre-gamma-fold)
        bq0_ps = mm_ps()
        for k in range(KT):
            nc.tensor.matmul(bq0_ps[0:P, 0:1], wqkT[:, k, :],
                             bTh[:, k:k + 1], start=(k == 0), stop=(k == KT - 1))
        bq1_ps = mm_ps()
        for k in range(KT):
            nc.tensor.matmul(bq1_ps[0:DA, 0:1], wvT[:, k, :],
                             bTh[:, k:k + 1], start=(k == 0), stop=(k == KT - 1))
        bqk_col = persist.tile([P, 1], F32, tag="bqk_col")
        nc.vector.tensor_copy(out=bqk_col, in_=bq0_ps[0:P, 0:1])
        bv_col = persist.tile([DA, 1], F32, tag="bv_col")
        nc.vector.tensor_copy(out=bv_col, in_=bq1_ps[0:DA, 0:1])
        for k in range(KT):
            nc.vector.tensor_scalar_mul(out=wqkT[:, k, :], in0=wqkT[:, k, :],
                                        scalar1=gT[:, k:k + 1])
            nc.vector.tensor_scalar_mul(out=wvT[:, k, :], in0=wvT[:, k, :],
                                        scalar1=gT[:, k:k + 1])

        # ---------------- persistent activations ----------------
        zTg = [persist.tile([P, KT, 512], BF16, tag=f"zTg{g}", name=f"zTg{g}")
               for g in range(NG)]
        qT = persist.tile([DA, SOWN], BF16, tag="qT")
        kTt = persist.tile([DA, SFULL], BF16, tag="kTt")
        vT = persist.tile([DA, SFULL], BF16, tag="vT")
        v_nat = persist.tile([P, ST, DA], BF16, tag="v_nat")
        v_aug = persist.tile([P, NH, ST, HD + 1], BF16, tag="v_aug")
        aoTn = persist.tile([DA, SOWN], BF16, tag="aoTn")
        aopT = persist.tile([P, KT, SOWN], BF16, tag="aopT")
        t8 = persist.tile([R, SOWN], BF16, tag="t8")
        mu_row = persist.tile([1, SOWN], BF16, tag="mu_row")
        mu_all = persist.tile([P, MT], F32, tag="mu_all")
        std_all = persist.tile([P, MT], F32, tag="std_all")

        nc.gpsimd.memset(v_aug[:, :, :, HD:HD + 1], 1.0)

        # ---------------- phase 1: layernorm + PE transposes ----------------
        for st in range(ST):
            own = st < MT
            src = x_own if own else x_oth
            row0 = st * P if own else (st - MT) * P
            xf = ldp.tile([P, E], F32, tag="xin")
            nc.gpsimd.dma_start(out=xf, in_=src[row0:row0 + P, :])

            stats = st_pool.tile([P, 2, 6], F32, tag="bnstats")
            xr = xf.rearrange("p (n f) -> p n f", f=512)
            for sg in range(2):
                nc.vector.bn_stats(out=stats[:, sg, :], in_=xr[:, sg, :])
            mv = st_pool.tile([P, 2], F32, tag="mv")
            nc.vector.bn_aggr(out=mv, in_=stats)

            # rstd = 1/sqrt(var+eps); nmr = -mu*rstd
            rstd = st_pool.tile([P, 1], F32, tag="rstd")
            nc.scalar.activation(out=rstd, in_=mv[:, 1:2], func=AF.Sqrt,
                                 bias=eps_t)
            if own:
                nc.vector.tensor_copy(out=std_all[:, st:st + 1], in_=rstd)
                nc.vector.tensor_copy(out=mu_all[:, st:st + 1], in_=mv[:, 0:1])
            nc.vector.reciprocal(out=rstd, in_=rstd)
            nmr = st_pool.tile([P, 1], F32, tag="nmr")
            nc.vector.tensor_scalar(out=nmr, in0=mv[:, 0:1], scalar1=rstd,
                                    scalar2=-1.0, op0=MUL, op1=MUL)
            # z = (x - mu) * rstd  (bf16); PE-transpose 8 k-tiles into one
            # [128, 1024] bf16 psum, evict wide into the zT group tensor
            zh = zh_pool.tile([P, E], BF16, tag="zh")
            nc.scalar.activation(out=zh, in_=xf, func=AF.Identity,
                                 scale=rstd, bias=nmr)
            ztp = tp_ps()
            for k in range(KT):
                nc.tensor.matmul(ztp[:, k * P:(k + 1) * P],
                                 zh[:, k * P:(k + 1) * P], identh,
                                 is_transpose=True)
            g, sub = divmod(st, 4)
            nc.vector.tensor_copy(
                out=zTg[g][:, :, sub * P:(sub + 1) * P],
                in_=ztp.rearrange("p (k c) -> p k c", c=P))

        # mu_row[0, m] = mu[m]: psum[0, j] = sum_p mu[p] * I[p, j]
        mu_allh = persist.tile([P, MT], BF16, tag="mu_allh")
        nc.vector.tensor_copy(out=mu_allh, in_=mu_all)
        for half in range(2):
            mur_ps = mm_ps()
            for mh in range(4):
                mt = half * 4 + mh
                nc.tensor.matmul(mur_ps[0:1, mh * P:(mh + 1) * P],
                                 mu_allh[:, mt:mt + 1], identh[:, 0:P],
                                 start=True, stop=True)
            nc.vector.tensor_copy(
                out=mu_row[:, half * 512:(half + 1) * 512],
                in_=mur_ps[0:1, :])

        # ---------------- phase 2: qkv projections ----------------
        for g in range(NG):
            pqk = mm_ps()
            for k in range(KT):
                nc.tensor.matmul(pqk, wqkT[:, k, :], zTg[g][:, k, :],
                                 start=(k == 0), stop=(k == KT - 1))
            nc.vector.tensor_scalar_add(
                out=kTt[:, g * 512:(g + 1) * 512],
                in0=pqk[DA:P, :], scalar1=bqk_col[DA:P, :])
            if g < 2:
                nc.vector.tensor_scalar_add(
                    out=qT[:, g * 512:(g + 1) * 512],
                    in0=pqk[0:DA, :], scalar1=bqk_col[0:DA, :])
        for g in range(NG):
            pv = mm_ps()
            for k in range(KT):
                nc.tensor.matmul(pv[0:DA, :], wvT[:, k, :], zTg[g][:, k, :],
                                 start=(k == 0), stop=(k == KT - 1))
            nc.vector.tensor_scalar_add(
                out=vT[:, g * 512:(g + 1) * 512],
                in0=pv[0:DA, :], scalar1=bv_col)
        # v natural via contiguous-dest DMA transpose (ACT queue), then
        # per-head v_aug (col 32 of each head slot is the ones column)
        nc.scalar.dma_start_transpose(out=v_nat, in_=vT)
        nc.vector.tensor_copy(out=v_aug[:, 0, :, 0:HD], in_=v_nat[:, :, 0:HD])
        nc.vector.tensor_copy(out=v_aug[:, 1, :, 0:HD], in_=v_nat[:, :, HD:DA])

        # ---------------- phase 3: attention ----------------
        CH = 8  # jt per chunk
        for h in range(NH):
            d0 = h * HD
            pav = [av_ps() for _ in range(2)]
            for chunk in range(ST // CH):
                pTc = pt_pool.tile([P, CH, SOWN], BF16, tag="pT")
                for j8 in range(CH):
                    jt = chunk * CH + j8
                    for mgrp in range(2):
                        psc = mm_ps()
                        nc.tensor.matmul(
                            psc,
                            kTt[d0:d0 + HD, jt * P:(jt + 1) * P],
                            qT[d0:d0 + HD, mgrp * 512:(mgrp + 1) * 512],
                            start=True, stop=True)
                        nc.scalar.activation(
                            out=pTc[:, j8, mgrp * 512:(mgrp + 1) * 512],
                            in_=psc, func=AF.Exp, scale=ATT_SCALE)
                for j8 in range(CH):
                    jt = chunk * CH + j8
                    for mgrp in range(2):
                        nc.tensor.matmul(
                            pav[mgrp], v_aug[:, h, jt, :],
                            pTc[:, j8, mgrp * 512:(mgrp + 1) * 512],
                            start=(jt == 0), stop=(jt == ST - 1))
            # normalize head h: broadcast 1/denominator via K=1 ones matmul
            for mgrp in range(2):
                rr = st_pool.tile([1, 512], F32, tag="rr", bufs=2)
                nc.vector.reciprocal(out=rr, in_=pav[mgrp][HD:HD + 1, :])
                rrh = st_pool.tile([1, 512], BF16, tag="rrh", bufs=2)
                nc.vector.tensor_copy(out=rrh, in_=rr)
                rrb_ps = av_ps()
                nc.tensor.matmul(rrb_ps[0:HD, :], ones_row[0:1, 0:HD],
                                 rrh, start=True, stop=True)
                rrb_sb = st_pool.tile([HD, 512], F32, tag="rrb_sb", bufs=2)
                nc.vector.tensor_copy(out=rrb_sb, in_=rrb_ps[0:HD, :])
                nc.vector.tensor_tensor(
                    out=aoTn[d0:d0 + HD, mgrp * 512:(mgrp + 1) * 512],
                    in0=pav[mgrp][0:HD, :], in1=rrb_sb, op=MUL)

        # ---------------- phase 4: attn_out projection ----------------
        for et in range(KT):
            for mgrp in range(2):
                pp = mm_ps()
                nc.tensor.matmul(pp, waoT[:, et, :],
                                 aoTn[:, mgrp * 512:(mgrp + 1) * 512],
                                 start=True, stop=True)
                nc.vector.tensor_copy(
                    out=aopT[:, et, mgrp * 512:(mgrp + 1) * 512], in_=pp)

        # ---------------- phase 5: lora down ----------------
        # t[r, m] = sum_e (g*ld)[e,r] z[e,m] + ld[e,r] aop[e,m] + bld[r]
        for mgrp in range(2):
            p5 = av_ps()
            for k in range(KT):
                nc.tensor.matmul(p5[0:R, :], ld_g[:, k, :], zTg[mgrp][:, k, :],
                                 start=(k == 0), stop=False)
            for k in range(KT):
                nc.tensor.matmul(p5[0:R, :], ld_raw[:, k, :],
                                 aopT[:, k, mgrp * 512:(mgrp + 1) * 512],
                                 start=False, stop=(k == KT - 1))
            nc.vector.tensor_scalar_add(
                out=t8[:, mgrp * 512:(mgrp + 1) * 512],
                in0=p5[0:R, :], scalar1=bld)

        # ---------------- phase 6: base + lora up + mu correction ----------------
        # out = std[m]*(z@Wb^T) + (t@lu/8 + mu x wbsum)
        for mt in range(MT):
            g, sub = divmod(mt, 4)
            o_t = o_pool.tile([P, DM], F32, tag="o_t")
            for grp in range(2):
                pA = mm_ps()
                for k in range(KT):
                    nc.tensor.matmul(pA,
                                     zTg[g][:, k, sub * P:(sub + 1) * P],
                                     WbT[:, k, grp * 512:(grp + 1) * 512],
                                     start=(k == 0), stop=(k == KT - 1))
                pB = mm_ps()
                nc.tensor.matmul(pB, t8[:, mt * P:(mt + 1) * P],
                                 lu8[:, grp * 512:(grp + 1) * 512],
                                 start=True, stop=False)
                nc.tensor.matmul(pB, mu_row[:, mt * P:(mt + 1) * P],
                                 wbs_sb[:, grp * 512:(grp + 1) * 512],
                                 start=False, stop=True)
                nc.scalar.activation(out=o_t[:, grp * 512:(grp + 1) * 512],
                                     in_=pA, func=AF.Copy,
                                     scale=std_all[:, mt:mt + 1])
                nc.vector.tensor_tensor(
                    out=o_t[:, grp * 512:(grp + 1) * 512],
                    in0=o_t[:, grp * 512:(grp + 1) * 512],
                    in1=pB, op=mybir.AluOpType.add)
                nc.sync.dma_start(
                    out=out_d[mt * P:(mt + 1) * P, grp * 512:(grp + 1) * 512],
                    in_=o_t[:, grp * 512:(grp + 1) * 512])

    nc.compile()
    return nc


_NC_CACHE = None


def _get_nc():
    global _NC_CACHE
    if _NC_CACHE is None:
        _NC_CACHE = build_kernel()
    return _NC_CACHE


def kernel(x, w_base, ln_gamma, ln_beta, lora_down, lora_up, w_qkv, w_attn_out,
           _trace=False):
    x = np.ascontiguousarray(np.asarray(x, dtype=np.float32))
    wk = {
        "w_base": np.ascontiguousarray(np.asarray(w_base, np.float32)),
        "ln_g": np.ascontiguousarray(np.asarray(ln_gamma, np.float32)),
        "ln_b": np.ascontiguousarray(np.asarray(ln_beta, np.float32)),
        "ld": np.ascontiguousarray(np.asarray(lora_down, np.float32)),
        "lu": np.ascontiguousarray(np.asarray(lora_up, np.float32)),
        "w_qkv": np.ascontiguousarray(np.asarray(w_qkv, np.float32)),
        "w_ao": np.ascontiguousarray(np.asarray(w_attn_out, np.float32)),
    }
    nc = _get_nc()
    in_maps = []
    for c in range(NC):
        b, half = divmod(c, 2)
        own = np.ascontiguousarray(x[b, half * SOWN:(half + 1) * SOWN])
        oth = np.ascontiguousarray(x[b, (1 - half) * SOWN:(2 - half) * SOWN])
        in_maps.append({"x_own": own, "x_oth": oth, **wk})
    res = run_bass_kernel_spmd(nc, in_maps, core_ids=list(range(NC)), trace=_trace)
    B, S = x.shape[0], x.shape[1]
    out = np.empty((B, S, DM), np.float32)
    for c in range(NC):
        b, half = divmod(c, 2)
        out[b, half * SOWN:(half + 1) * SOWN] = res.results[c]["out"]
    if _trace:
        kernel.last_exec_time_ns = res.exec_time_ns
        kernel.last_results = res
    return out


# revision 14
# speedup vs baseline: 1.7567x; 1.2908x over previous
"""Trainium2 Bass kernel for the LoRA-BC block (nn_LoRABCBlock).

Computation (per reference):
    base = x @ w_base.T
    h = layernorm(x) * gamma + beta
    qkv = h @ w_qkv.T ; attention (2 heads, head_dim 32) over full sequence
    attn_out = attn_output @ w_attn_out.T
    delta = ((h + attn_out) @ lora_down) @ lora_up
    out = base + (1/8) * delta

Sharding: data-parallel over (batch, seq-half) -> 8 cores. Each core owns
1024 query rows of one batch element, computes k/v over that batch's full
2048-row sequence, and produces its 1024 output rows. Weights replicated.
No collectives.

Structure (v5):
  - All transposes on the PE array, batched: 8x [128,128] transposes into
    one [128,1024] bf16 psum, one wide DVE evict. (The DMA XBAR measured
    ~7us per 256KB tile serialized on one queue - too slow.)
  - x tile loads are prefetched on two DMA rings (gpsimd + SP) ahead of
    the weight loads so layernorm starts immediately.
  - gamma/beta folded into w_qkv / lora_down + bias columns. The base
    matmul runs on zT directly: out = std[m]*(z@Wb^T) + mu x wbsum + delta,
    with a per-partition std scale at eviction; the rank-8 LoRA + rank-1
    mu terms accumulate in a second psum added at eviction.
  - Attention computes transposed scores (scoresT[j,m]); probabilities
    come out in the layout attn@v needs. Per head, 8-jt chunks of
    scores+exp are batched, then attn@v runs back-to-back PE matmuls into
    per-head [33,512] accumulators whose row 32 is the softmax denominator
    (an appended ones-column of V). Denominator reciprocals are broadcast
    across partitions with K=1 ones matmuls.
"""

import sys

sys.path.insert(0, "/opt/trn_rl_repo")

from contextlib import ExitStack

import numpy as np

import concourse.bass as bass
import concourse.tile as tile
from concourse import bacc, mybir
from concourse.bass_utils import run_bass_kernel_spmd
from concourse.masks import make_identity

F32 = mybir.dt.float32
BF16 = mybir.dt.bfloat16
AF = mybir.ActivationFunctionType
MUL = mybir.AluOpType.mult

E = 1024          # embed dim
DM = 1024         # d_model
R = 8             # lora rank
SCALING = 1.0 / R
DA = 64           # attn dim
NH = 2            # heads
HD = DA // NH     # head dim = 32
SOWN = 1024       # rows owned per core
SFULL = 2048      # rows per batch element
NC = 8            # cores
P = 128
KT = E // P       # 8 k-tiles
MT = SOWN // P    # 8 own m-tiles
ST = SFULL // P   # 16 sequence tiles
NG = SFULL // 512  # 4 zT column groups
ATT_SCALE = float(HD) ** -0.5
XPRE = 6          # x tiles prefetched ahead of weight loads


def build_kernel():
    nc = bacc.Bacc("TRN2", target_bir_lowering=False, debug=False, num_devices=NC)

    x_own = nc.dram_tensor("x_own", [SOWN, E], F32, kind="ExternalInput").ap()
    x_oth = nc.dram_tensor("x_oth", [SOWN, E], F32, kind="ExternalInput").ap()
    w_base = nc.dram_tensor("w_base", [DM, E], F32, kind="ExternalInput").ap()
    ln_g = nc.dram_tensor("ln_g", [E], F32, kind="ExternalInput").ap()
    ln_b = nc.dram_tensor("ln_b", [E], F32, kind="ExternalInput").ap()
    ld = nc.dram_tensor("ld", [E, R], F32, kind="ExternalInput").ap()
    lu = nc.dram_tensor("lu", [R, DM], F32, kind="ExternalInput").ap()
    w_qkv = nc.dram_tensor("w_qkv", [3 * DA, E], F32, kind="ExternalInput").ap()
    w_ao = nc.dram_tensor("w_ao", [E, DA], F32, kind="ExternalInput").ap()
    out_d = nc.dram_tensor("out", [SOWN, DM], F32, kind="ExternalOutput").ap()

    def xsrc(st):
        own = st < MT
        src = x_own if own else x_oth
        row0 = st * P if own else (st - MT) * P
        return src[row0:row0 + P, :]

    with tile.TileContext(nc) as tc, ExitStack() as ctx:
        persist = ctx.enter_context(tc.tile_pool(name="persist", bufs=1))
        ldp = ctx.enter_context(tc.tile_pool(name="loads", bufs=3))
        zh_pool = ctx.enter_context(tc.tile_pool(name="zh", bufs=3))
        st_pool = ctx.enter_context(tc.tile_pool(name="stats", bufs=4))
        pt_pool = ctx.enter_context(tc.tile_pool(name="pt", bufs=2))
        o_pool = ctx.enter_context(tc.tile_pool(name="outs", bufs=2))
        ps = ctx.enter_context(tc.tile_pool(name="ps", bufs=1, space="PSUM"))

        _n = [0]

        def ps_tile(shape, dtype, tag, bufs):
            _n[0] += 1
            return ps.tile(shape, dtype, tag=tag, bufs=bufs,
                           name=f"ps_{tag}_{_n[0]}")

        def mm_ps():
            return ps_tile([P, 512], F32, "mm", 3)

        def av_ps():
            return ps_tile([HD + 1, 512], F32, "av", 3)

        def tp_ps():
            return ps_tile([P, E], BF16, "tp", 2)

        # ---------------- constants ----------------
        identh = persist.tile([P, P], BF16, tag="identh")
        make_identity(nc, identh)

        eps_t = persist.tile([P, 1], F32, tag="eps")
        nc.vector.memset(eps_t, 1e-5)
        ones_col = persist.tile([P, 1], BF16, tag="ones_col")
        nc.vector.memset(ones_col, 1.0)
        ones_row = persist.tile([1, DA], BF16, tag="ones_row")
        nc.vector.memset(ones_row, 1.0)

        # ---------------- x prefetch (2 DMA rings) ----------------
        xfs = {}
        for st in range(XPRE):
            xf = ldp.tile([P, E], F32, tag="xin", bufs=XPRE, name=f"xf{st}")
            eng = nc.gpsimd if st % 2 == 0 else nc.sync
            eng.dma_start(out=xf, in_=xsrc(st))
            xfs[st] = xf

        # gamma/beta arranged [p, kt] (e = kt*128 + p)
        gT = persist.tile([P, KT], F32, tag="gT")
        bT = persist.tile([P, KT], F32, tag="bT")
        nc.sync.dma_start(out=gT, in_=ln_g.rearrange("(kt p) -> p kt", p=P))
        nc.sync.dma_start(out=bT, in_=ln_b.rearrange("(kt p) -> p kt", p=P))
        bTh = persist.tile([P, KT], BF16, tag="bTh")
        nc.vector.tensor_copy(out=bTh, in_=bT)

        def pe_transpose_128(dst, src_h, nk, rows=P):
            """Transpose nk [rows,128] column blocks of src_h via PE into one
            bf16 psum, then one wide DVE evict into dst ([rows*? see callers])."""
            tp = tp_ps()
            for k in range(nk):
                nc.tensor.matmul(tp[0:P, k * rows:(k + 1) * rows],
                                 src_h[:, k * P:(k + 1) * P],
                                 identh[0:src_h.shape[0], :],
                                 is_transpose=True)
            return tp

        # w_qkv -> wqkT [p, kt, 128] and wvT [p, kt, 64]
        wqkT = persist.tile([P, KT, P], BF16, tag="wqkT")
        wvT = persist.tile([P, KT, DA], BF16, tag="wvT")
        wq0 = ldp.tile([P, E], F32, tag="wload", bufs=3)
        nc.gpsimd.dma_start(out=wq0, in_=w_qkv[0:P, :])
        wq0h = zh_pool.tile([P, E], BF16, tag="zh")
        nc.vector.tensor_copy(out=wq0h, in_=wq0)
        tp = tp_ps()
        for k in range(KT):
            nc.tensor.matmul(tp[:, k * P:(k + 1) * P],
                             wq0h[:, k * P:(k + 1) * P], identh,
                             is_transpose=True)
        nc.vector.tensor_copy(out=wqkT, in_=tp.rearrange("p (k c) -> p k c", c=P))

        wq1t = ldp.tile([P, E], F32, tag="wload", bufs=3)
        wq1 = wq1t[0:DA, :]
        nc.gpsimd.dma_start(out=wq1, in_=w_qkv[P:3 * DA, :])
        wq1h_t = zh_pool.tile([P, E], BF16, tag="zh")
        wq1h = wq1h_t[0:DA, :]
        nc.vector.tensor_copy(out=wq1h, in_=wq1)
        tp = tp_ps()
        for k in range(KT):
            nc.tensor.matmul(tp[:, k * DA:(k + 1) * DA],
                             wq1h[:, k * P:(k + 1) * P], identh[0:DA, 0:DA],
                             is_transpose=True)
        nc.vector.tensor_copy(out=wvT,
                              in_=tp[:, 0:KT * DA].rearrange("p (k c) -> p k c", c=DA))

        # w_attn_out [e, d] -> waoT [d, kt, p]
        waot = ldp.tile([P, E], F32, tag="wload", bufs=3)
        waof = waot[:, 0:KT * DA].rearrange("p (k d) -> p k d", d=DA)
        nc.gpsimd.dma_start(
            out=waof, in_=w_ao.rearrange("(kt p) d -> p kt d", p=P))
        waoh_t = zh_pool.tile([P, E], BF16, tag="zh")
        waoh = waoh_t[:, 0:KT * DA]
        nc.vector.tensor_copy(out=waoh, in_=waof.rearrange("p k d -> p (k d)"))
        tp = tp_ps()
        for k in range(KT):
            nc.tensor.matmul(tp[0:DA, k * P:(k + 1) * P],
                             waoh[:, k * DA:(k + 1) * DA], identh,
                             is_transpose=True)
        waoT = persist.tile([DA, KT, P], BF16, tag="waoT")
        nc.vector.tensor_copy(out=waoT,
                              in_=tp[0:DA, :].rearrange("p (k c) -> p k c", c=P))

        # w_base -> WbT[p, kt, n]
        WbT = persist.tile([P, KT, DM], BF16, tag="WbT")
        for nt in range(KT):
            wf = ldp.tile([P, E], F32, tag="wload", bufs=3)
            nc.gpsimd.dma_start(out=wf, in_=w_base[nt * P:(nt + 1) * P, :])
            wh = zh_pool.tile([P, E], BF16, tag="zh")
            nc.vector.tensor_copy(out=wh, in_=wf)
            tp = tp_ps()
            for k in range(KT):
                nc.tensor.matmul(tp[:, k * P:(k + 1) * P],
                                 wh[:, k * P:(k + 1) * P], identh,
                                 is_transpose=True)
            nc.vector.tensor_copy(
                out=WbT[:, :, nt * P:(nt + 1) * P],
                in_=tp.rearrange("p (k c) -> p k c", c=P))

        # wbsum[n] = sum_e w_base[n, e]
        wbs_sb = persist.tile([1, DM], BF16, tag="wbs_sb")
        for grp in range(2):
            wbs_ps = mm_ps()
            for k in range(KT):
                nc.tensor.matmul(wbs_ps[0:1, :], ones_col,
                                 WbT[:, k, grp * 512:(grp + 1) * 512],
                                 start=(k == 0), stop=(k == KT - 1))
            nc.vector.tensor_copy(out=wbs_sb[:, grp * 512:(grp + 1) * 512],
                                  in_=wbs_ps[0:1, :])

        # lora_up scaled
        lu_t = ldp.tile([P, E], F32, tag="wload", bufs=3)
        lu_f = lu_t[0:R, :]
        nc.gpsimd.dma_start(out=lu_f, in_=lu)
        lu8 = persist.tile([R, DM], BF16, tag="lu8")
        nc.scalar.mul(lu8, lu_f, SCALING)

        # lora_down [E, R] -> [p, kt, r]; gamma-folded + raw copies
        ld_t = ldp.tile([P, E], F32, tag="wload", bufs=3)
        ld_f = ld_t[:, 0:KT * R].rearrange("p (k r) -> p k r", r=R)
        nc.gpsimd.dma_start(out=ld_f, in_=ld.rearrange("(kt p) r -> p kt r", p=P))
        ld_raw = persist.tile([P, KT, R], BF16, tag="ld_raw")
        nc.vector.tensor_copy(out=ld_raw, in_=ld_f)
        ld_g = persist.tile([P, KT, R], BF16, tag="ld_g")
        for k in range(KT):
            nc.vector.tensor_scalar_mul(out=ld_g[:, k, :], in0=ld_f[:, k, :],
                                        scalar1=gT[:, k:k + 1])
        # bld[r] = sum_e beta[e] * lora_down[e, r]
        bld_ps = av_ps()
        for k in range(KT):
            nc.tensor.matmul(bld_ps[0:R, 0:1], ld_raw[:, k, :], bTh[:, k:k + 1],
                             start=(k == 0), stop=(k == KT - 1))
        bld = persist.tile([R, 1], F32, tag="bld")
        nc.vector.tensor_copy(out=bld, in_=bld_ps[0:R, 0:1])

        # qkv bias columns from beta (raw weights, pre-gamma-fold)
        bq0_ps = mm_ps()
        for k in range(KT):
            nc.tensor.matmul(bq0_ps[0:P, 0:1], wqkT[:, k, :],
                             bTh[:, k:k + 1], start=(k == 0), stop=(k == KT - 1))
        bq1_ps = mm_ps()
        for k in range(KT):
            nc.tensor.matmul(bq1_ps[0:DA, 0:1], wvT[:, k, :],
                             bTh[:, k:k + 1], start=(k == 0), stop=(k == KT - 1))
        bqk_col = persist.tile([P, 1], F32, tag="bqk_col")
        nc.vector.tensor_copy(out=bqk_col, in_=bq0_ps[0:P, 0:1])
        bv_col = persist.tile([DA, 1], F32, tag="bv_col")
        nc.vector.tensor_copy(out=bv_col, in_=bq1_ps[0:DA, 0:1])
        for k in range(KT):
            nc.vector.tensor_scalar_mul(out=wqkT[:, k, :], in0=wqkT[:, k, :],
                                        scalar1=gT[:, k:k + 1])
            nc.vector.tensor_scalar_mul(out=wvT[:, k, :], in0=wvT[:, k, :],
                                        scalar1=gT[:, k:k + 1])

        # ---------------- persistent activations ----------------
        zTg = [persist.tile([P, KT, 512], BF16, tag=f"zTg{g}", name=f"zTg{g}")
               for g in range(NG)]
        qT = persist.tile([DA, SOWN], BF16, tag="qT")
        kTt = persist.tile([DA, SFULL], BF16, tag="kTt")
        vT = persist.tile([DA, SFULL], BF16, tag="vT")
        v_nat = persist.tile([P, ST, DA], BF16, tag="v_nat")
        v_aug = persist.tile([P, NH, ST, HD + 1], BF16, tag="v_aug")
        aoTn = persist.tile([DA, SOWN], BF16, tag="aoTn")
        aopT = persist.tile([P, KT, SOWN], BF16, tag="aopT")
        t8 = persist.tile([R, SOWN], BF16, tag="t8")
        mu_row = persist.tile([1, SOWN], BF16, tag="mu_row")
        mu_all = persist.tile([P, MT], F32, tag="mu_all")
        std_all = persist.tile([P, MT], F32, tag="std_all")

        nc.gpsimd.memset(v_aug[:, :, :, HD:HD + 1], 1.0)

        # ---------------- phase 1: layernorm + PE transposes ----------------
        for st in range(ST):
            own = st < MT
            if st in xfs:
                xf = xfs[st]
            else:
                xf = ldp.tile([P, E], F32, tag="xin", bufs=XPRE)
                eng = nc.gpsimd if st % 2 == 0 else nc.sync
                eng.dma_start(out=xf, in_=xsrc(st))

            stats = st_pool.tile([P, 2, 6], F32, tag="bnstats")
            xr = xf.rearrange("p (n f) -> p n f", f=512)
            for sg in range(2):
                nc.vector.bn_stats(out=stats[:, sg, :], in_=xr[:, sg, :])
            mv = st_pool.tile([P, 2], F32, tag="mv")
            nc.vector.bn_aggr(out=mv, in_=stats)

            rstd = st_pool.tile([P, 1], F32, tag="rstd")
            nc.scalar.activation(out=rstd, in_=mv[:, 1:2], func=AF.Sqrt,
                                 bias=eps_t)
            if own:
                nc.vector.tensor_copy(out=std_all[:, st:st + 1], in_=rstd)
                nc.vector.tensor_copy(out=mu_all[:, st:st + 1], in_=mv[:, 0:1])
            nc.vector.reciprocal(out=rstd, in_=rstd)
            nmr = st_pool.tile([P, 1], F32, tag="nmr")
            nc.vector.tensor_scalar(out=nmr, in0=mv[:, 0:1], scalar1=rstd,
                                    scalar2=-1.0, op0=MUL, op1=MUL)
            zh = zh_pool.tile([P, E], BF16, tag="zh")
            nc.scalar.activation(out=zh, in_=xf, func=AF.Identity,
                                 scale=rstd, bias=nmr)
            ztp = tp_ps()
            for k in range(KT):
                nc.tensor.matmul(ztp[:, k * P:(k + 1) * P],
                                 zh[:, k * P:(k + 1) * P], identh,
                                 is_transpose=True)
            g, sub = divmod(st, 4)
            nc.vector.tensor_copy(
                out=zTg[g][:, :, sub * P:(sub + 1) * P],
                in_=ztp.rearrange("p (k c) -> p k c", c=P))

        # mu_row[0, m] = mu[m]
        mu_allh = persist.tile([P, MT], BF16, tag="mu_allh")
        nc.vector.tensor_copy(out=mu_allh, in_=mu_all)
        for half in range(2):
            mur_ps = mm_ps()
            for mh in range(4):
                mt = half * 4 + mh
                nc.tensor.matmul(mur_ps[0:1, mh * P:(mh + 1) * P],
                                 mu_allh[:, mt:mt + 1], identh[:, 0:P],
                                 start=True, stop=True)
            nc.vector.tensor_copy(
                out=mu_row[:, half * 512:(half + 1) * 512],
                in_=mur_ps[0:1, :])

        # ---------------- phase 2: qkv projections ----------------
        for g in range(NG):
            pqk = mm_ps()
            for k in range(KT):
                nc.tensor.matmul(pqk, wqkT[:, k, :], zTg[g][:, k, :],
                                 start=(k == 0), stop=(k == KT - 1))
            nc.vector.tensor_scalar_add(
                out=kTt[:, g * 512:(g + 1) * 512],
                in0=pqk[DA:P, :], scalar1=bqk_col[DA:P, :])
            if g < 2:
                nc.vector.tensor_scalar_add(
                    out=qT[:, g * 512:(g + 1) * 512],
                    in0=pqk[0:DA, :], scalar1=bqk_col[0:DA, :])
        for g in range(NG):
            pv = mm_ps()
            for k in range(KT):
                nc.tensor.matmul(pv[0:DA, :], wvT[:, k, :], zTg[g][:, k, :],
                                 start=(k == 0), stop=(k == KT - 1))
            nc.vector.tensor_scalar_add(
                out=vT[:, g * 512:(g + 1) * 512],
                in0=pv[0:DA, :], scalar1=bv_col)
        # v natural via PE transposes: vT [64, jt*128] -> v_nat [p, jt, 64]
        vtp = tp_ps()
        for jt in range(ST):
            nc.tensor.matmul(vtp[:, jt * DA:(jt + 1) * DA],
                             vT[:, jt * P:(jt + 1) * P], identh[0:DA, 0:DA],
                             is_transpose=True)
        nc.vector.tensor_copy(out=v_nat,
                              in_=vtp.rearrange("p (j d) -> p j d", d=DA))
        nc.vector.tensor_copy(out=v_aug[:, 0, :, 0:HD], in_=v_nat[:, :, 0:HD])
        nc.vector.tensor_copy(out=v_aug[:, 1, :, 0:HD], in_=v_nat[:, :, HD:DA])

        # ---------------- phase 3: attention ----------------
        CH = 8  # jt per chunk
        for h in range(NH):
            d0 = h * HD
            pav = [av_ps() for _ in range(2)]
            for chunk in range(ST // CH):
                pTc = pt_pool.tile([P, CH, SOWN], BF16, tag="pT")
                for j8 in range(CH):
                    jt = chunk * CH + j8
                    for mgrp in range(2):
                        psc = mm_ps()
                        nc.tensor.matmul(
                            psc,
                            kTt[d0:d0 + HD, jt * P:(jt + 1) * P],
                            qT[d0:d0 + HD, mgrp * 512:(mgrp + 1) * 512],
                            start=True, stop=True)
                        nc.scalar.activation(
                            out=pTc[:, j8, mgrp * 512:(mgrp + 1) * 512],
                            in_=psc, func=AF.Exp, scale=ATT_SCALE)
                for j8 in range(CH):
                    jt = chunk * CH + j8
                    for mgrp in range(2):
                        nc.tensor.matmul(
                            pav[mgrp], v_aug[:, h, jt, :],
                            pTc[:, j8, mgrp * 512:(mgrp + 1) * 512],
                            start=(jt == 0), stop=(jt == ST - 1))
            for mgrp in range(2):
                rr = st_pool.tile([1, 512], F32, tag="rr", bufs=2)
                nc.vector.reciprocal(out=rr, in_=pav[mgrp][HD:HD + 1, :])
                rrh = st_pool.tile([1, 512], BF16, tag="rrh", bufs=2)
                nc.vector.tensor_copy(out=rrh, in_=rr)
                rrb_ps = av_ps()
                nc.tensor.matmul(rrb_ps[0:HD, :], ones_row[0:1, 0:HD],
                                 rrh, start=True, stop=True)
                rrb_sb = st_pool.tile([HD, 512], F32, tag="rrb_sb", bufs=2)
                nc.vector.tensor_copy(out=rrb_sb, in_=rrb_ps[0:HD, :])
                nc.vector.tensor_tensor(
                    out=aoTn[d0:d0 + HD, mgrp * 512:(mgrp + 1) * 512],
                    in0=pav[mgrp][0:HD, :], in1=rrb_sb, op=MUL)

        # ---------------- phase 4: attn_out projection ----------------
        for et in range(KT):
            for mgrp in range(2):
                pp = mm_ps()
                nc.tensor.matmul(pp, waoT[:, et, :],
                                 aoTn[:, mgrp * 512:(mgrp + 1) * 512],
                                 start=True, stop=True)
                nc.vector.tensor_copy(
                    out=aopT[:, et, mgrp * 512:(mgrp + 1) * 512], in_=pp)

        # ---------------- phase 5: lora down ----------------
        for mgrp in range(2):
            p5 = av_ps()
            for k in range(KT):
                nc.tensor.matmul(p5[0:R, :], ld_g[:, k, :], zTg[mgrp][:, k, :],
                                 start=(k == 0), stop=False)
            for k in range(KT):
                nc.tensor.matmul(p5[0:R, :], ld_raw[:, k, :],
                                 aopT[:, k, mgrp * 512:(mgrp + 1) * 512],
                                 start=False, stop=(k == KT - 1))
            nc.vector.tensor_scalar_add(
                out=t8[:, mgrp * 512:(mgrp + 1) * 512],
                in0=p5[0:R, :], scalar1=bld)

        # ---------------- phase 6: base + lora up + mu correction ----------------
        # out = std[m]*(z@Wb^T) + (t@lu/8 + mu x wbsum)
        for mt in range(MT):
            g, sub = divmod(mt, 4)
            o_t = o_pool.tile([P, DM], F32, tag="o_t")
            for grp in range(2):
                pA = mm_ps()
                for k in range(KT):
                    nc.tensor.matmul(pA,
                                     zTg[g][:, k, sub * P:(sub + 1) * P],
                                     WbT[:, k, grp * 512:(grp + 1) * 512],
                                     start=(k == 0), stop=(k == KT - 1))
                pB = mm_ps()
                nc.tensor.matmul(pB, t8[:, mt * P:(mt + 1) * P],
                                 lu8[:, grp * 512:(grp + 1) * 512],
                                 start=True, stop=False)
                nc.tensor.matmul(pB, mu_row[:, mt * P:(mt + 1) * P],
                                 wbs_sb[:, grp * 512:(grp + 1) * 512],
                                 start=False, stop=True)
                nc.scalar.activation(out=o_t[:, grp * 512:(grp + 1) * 512],
                                     in_=pA, func=AF.Copy,
                                     scale=std_all[:, mt:mt + 1])
                nc.vector.tensor_tensor(
                    out=o_t[:, grp * 512:(grp + 1) * 512],
                    in0=o_t[:, grp * 512:(grp + 1) * 512],
                    in1=pB, op=mybir.AluOpType.add)
                nc.sync.dma_start(
                    out=out_d[mt * P:(mt + 1) * P, grp * 512:(grp + 1) * 512],
                    in_=o_t[:, grp * 512:(grp + 1) * 512])

    nc.compile()
    return nc


_NC_CACHE = None


def _get_nc():
    global _NC_CACHE
    if _NC_CACHE is None:
        _NC_CACHE = build_kernel()
    return _NC_CACHE


def kernel(x, w_base, ln_gamma, ln_beta, lora_down, lora_up, w_qkv, w_attn_out,
           _trace=False):
    x = np.ascontiguousarray(np.asarray(x, dtype=np.float32))
    wk = {
        "w_base": np.ascontiguousarray(np.asarray(w_base, np.float32)),
        "ln_g": np.ascontiguousarray(np.asarray(ln_gamma, np.float32)),
        "ln_b": np.ascontiguousarray(np.asarray(ln_beta, np.float32)),
        "ld": np.ascontiguousarray(np.asarray(lora_down, np.float32)),
        "lu": np.ascontiguousarray(np.asarray(lora_up, np.float32)),
        "w_qkv": np.ascontiguousarray(np.asarray(w_qkv, np.float32)),
        "w_ao": np.ascontiguousarray(np.asarray(w_attn_out, np.float32)),
    }
    nc = _get_nc()
    in_maps = []
    for c in range(NC):
        b, half = divmod(c, 2)
        own = np.ascontiguousarray(x[b, half * SOWN:(half + 1) * SOWN])
        oth = np.ascontiguousarray(x[b, (1 - half) * SOWN:(2 - half) * SOWN])
        in_maps.append({"x_own": own, "x_oth": oth, **wk})
    res = run_bass_kernel_spmd(nc, in_maps, core_ids=list(range(NC)), trace=_trace)
    B, S = x.shape[0], x.shape[1]
    out = np.empty((B, S, DM), np.float32)
    for c in range(NC):
        b, half = divmod(c, 2)
        out[b, half * SOWN:(half + 1) * SOWN] = res.results[c]["out"]
    if _trace:
        kernel.last_exec_time_ns = res.exec_time_ns
        kernel.last_results = res
    return out


# revision 15
# speedup vs baseline: 1.8364x; 1.0453x over previous
"""Trainium2 Bass kernel for the LoRA-BC block (nn_LoRABCBlock).

Computation (per reference):
    base = x @ w_base.T
    h = layernorm(x) * gamma + beta
    qkv = h @ w_qkv.T ; attention (2 heads, head_dim 32) over full sequence
    attn_out = attn_output @ w_attn_out.T
    delta = ((h + attn_out) @ lora_down) @ lora_up
    out = base + (1/8) * delta

Sharding: data-parallel over (batch, seq-half) -> 8 cores. Each core owns
1024 query rows of one batch element, computes k/v over that batch's full
2048-row sequence, and produces its 1024 output rows. Weights replicated.
No collectives.

Structure (v5):
  - All transposes on the PE array, batched: 8x [128,128] transposes into
    one [128,1024] bf16 psum, one wide DVE evict. (The DMA XBAR measured
    ~7us per 256KB tile serialized on one queue - too slow.)
  - x tile loads are prefetched on two DMA rings (gpsimd + SP) ahead of
    the weight loads so layernorm starts immediately.
  - gamma/beta folded into w_qkv / lora_down + bias columns. The base
    matmul runs on zT directly: out = std[m]*(z@Wb^T) + mu x wbsum + delta,
    with a per-partition std scale at eviction; the rank-8 LoRA + rank-1
    mu terms accumulate in a second psum added at eviction.
  - Attention computes transposed scores (scoresT[j,m]); probabilities
    come out in the layout attn@v needs. Per head, 8-jt chunks of
    scores+exp are batched, then attn@v runs back-to-back PE matmuls into
    per-head [33,512] accumulators whose row 32 is the softmax denominator
    (an appended ones-column of V). Denominator reciprocals are broadcast
    across partitions with K=1 ones matmuls.
"""

import sys

sys.path.insert(0, "/opt/trn_rl_repo")

from contextlib import ExitStack

import numpy as np

import concourse.bass as bass
import concourse.tile as tile
from concourse import bacc, mybir
from concourse.bass_utils import run_bass_kernel_spmd
from concourse.masks import make_identity

F32 = mybir.dt.float32
BF16 = mybir.dt.bfloat16
AF = mybir.ActivationFunctionType
MUL = mybir.AluOpType.mult

E = 1024          # embed dim
DM = 1024         # d_model
R = 8             # lora rank
SCALING = 1.0 / R
DA = 64           # attn dim
NH = 2            # heads
HD = DA // NH     # head dim = 32
SOWN = 1024       # rows owned per core
SFULL = 2048      # rows per batch element
NC = 8            # cores
P = 128
KT = E // P       # 8 k-tiles
MT = SOWN // P    # 8 own m-tiles
ST = SFULL // P   # 16 sequence tiles
NG = SFULL // 512  # 4 zT column groups
ATT_SCALE = float(HD) ** -0.5
XPRE = 6          # x tiles prefetched ahead of weight loads


def build_kernel():
    nc = bacc.Bacc("TRN2", target_bir_lowering=False, debug=False, num_devices=NC)

    x_own = nc.dram_tensor("x_own", [SOWN, E], F32, kind="ExternalInput").ap()
    x_oth = nc.dram_tensor("x_oth", [SOWN, E], F32, kind="ExternalInput").ap()
    w_base = nc.dram_tensor("w_base", [DM, E], F32, kind="ExternalInput").ap()
    ln_g = nc.dram_tensor("ln_g", [E], F32, kind="ExternalInput").ap()
    ln_b = nc.dram_tensor("ln_b", [E], F32, kind="ExternalInput").ap()
    ld = nc.dram_tensor("ld", [E, R], F32, kind="ExternalInput").ap()
    lu = nc.dram_tensor("lu", [R, DM], F32, kind="ExternalInput").ap()
    w_qkv = nc.dram_tensor("w_qkv", [3 * DA, E], F32, kind="ExternalInput").ap()
    w_ao = nc.dram_tensor("w_ao", [E, DA], F32, kind="ExternalInput").ap()
    out_d = nc.dram_tensor("out", [SOWN, DM], F32, kind="ExternalOutput").ap()

    def xsrc(st):
        own = st < MT
        src = x_own if own else x_oth
        row0 = st * P if own else (st - MT) * P
        return src[row0:row0 + P, :]

    with tile.TileContext(nc) as tc, ExitStack() as ctx:
        persist = ctx.enter_context(tc.tile_pool(name="persist", bufs=1))
        ldp = ctx.enter_context(tc.tile_pool(name="loads", bufs=3))
        zh_pool = ctx.enter_context(tc.tile_pool(name="zh", bufs=3))
        st_pool = ctx.enter_context(tc.tile_pool(name="stats", bufs=4))
        pt_pool = ctx.enter_context(tc.tile_pool(name="pt", bufs=2))
        o_pool = ctx.enter_context(tc.tile_pool(name="outs", bufs=2))
        ps = ctx.enter_context(tc.tile_pool(name="ps", bufs=1, space="PSUM"))

        _n = [0]

        def ps_tile(shape, dtype, tag, bufs):
            _n[0] += 1
            return ps.tile(shape, dtype, tag=tag, bufs=bufs,
                           name=f"ps_{tag}_{_n[0]}")

        def mm_ps():
            return ps_tile([P, 512], F32, "mm", 3)

        def av_ps():
            return ps_tile([HD + 1, 512], F32, "av", 3)

        def tp_ps():
            return ps_tile([P, E], BF16, "tp", 2)

        def sc_ps():
            # scores psum reuses the (attention-idle) tp storage
            return ps_tile([P, 512], F32, "tp", 2)

        # ---------------- constants ----------------
        identh = persist.tile([P, P], BF16, tag="identh")
        make_identity(nc, identh)

        eps_t = persist.tile([P, 1], F32, tag="eps")
        nc.vector.memset(eps_t, 1e-5)
        ones_col = persist.tile([P, 1], BF16, tag="ones_col")
        nc.vector.memset(ones_col, 1.0)
        ones_row = persist.tile([1, DA], BF16, tag="ones_row")
        nc.vector.memset(ones_row, 1.0)

        # ---------------- x prefetch (2 DMA rings) ----------------
        xfs = {}
        for st in range(XPRE):
            xf = ldp.tile([P, E], F32, tag="xin", bufs=XPRE, name=f"xf{st}")
            eng = nc.gpsimd if st % 2 == 0 else nc.sync
            eng.dma_start(out=xf, in_=xsrc(st))
            xfs[st] = xf

        # gamma/beta arranged [p, kt] (e = kt*128 + p)
        gT = persist.tile([P, KT], F32, tag="gT")
        bT = persist.tile([P, KT], F32, tag="bT")
        nc.sync.dma_start(out=gT, in_=ln_g.rearrange("(kt p) -> p kt", p=P))
        nc.sync.dma_start(out=bT, in_=ln_b.rearrange("(kt p) -> p kt", p=P))
        bTh = persist.tile([P, KT], BF16, tag="bTh")
        nc.vector.tensor_copy(out=bTh, in_=bT)

        def pe_transpose_128(dst, src_h, nk, rows=P):
            """Transpose nk [rows,128] column blocks of src_h via PE into one
            bf16 psum, then one wide DVE evict into dst ([rows*? see callers])."""
            tp = tp_ps()
            for k in range(nk):
                nc.tensor.matmul(tp[0:P, k * rows:(k + 1) * rows],
                                 src_h[:, k * P:(k + 1) * P],
                                 identh[0:src_h.shape[0], :],
                                 is_transpose=True)
            return tp

        # w_qkv -> wqkT [p, kt, 128] and wvT [p, kt, 64]
        wqkT = persist.tile([P, KT, P], BF16, tag="wqkT")
        wvT = persist.tile([P, KT, DA], BF16, tag="wvT")
        wq0 = ldp.tile([P, E], F32, tag="wload", bufs=3)
        nc.gpsimd.dma_start(out=wq0, in_=w_qkv[0:P, :])
        wq0h = zh_pool.tile([P, E], BF16, tag="zh")
        nc.vector.tensor_copy(out=wq0h, in_=wq0)
        tp = tp_ps()
        for k in range(KT):
            nc.tensor.matmul(tp[:, k * P:(k + 1) * P],
                             wq0h[:, k * P:(k + 1) * P], identh,
                             is_transpose=True)
        nc.vector.tensor_copy(out=wqkT, in_=tp.rearrange("p (k c) -> p k c", c=P))

        wq1t = ldp.tile([P, E], F32, tag="wload", bufs=3)
        wq1 = wq1t[0:DA, :]
        nc.gpsimd.dma_start(out=wq1, in_=w_qkv[P:3 * DA, :])
        wq1h_t = zh_pool.tile([P, E], BF16, tag="zh")
        wq1h = wq1h_t[0:DA, :]
        nc.vector.tensor_copy(out=wq1h, in_=wq1)
        tp = tp_ps()
        for k in range(KT):
            nc.tensor.matmul(tp[:, k * DA:(k + 1) * DA],
                             wq1h[:, k * P:(k + 1) * P], identh[0:DA, 0:DA],
                             is_transpose=True)
        nc.vector.tensor_copy(out=wvT,
                              in_=tp[:, 0:KT * DA].rearrange("p (k c) -> p k c", c=DA))

        # w_attn_out [e, d] -> waoT [d, kt, p]
        waot = ldp.tile([P, E], F32, tag="wload", bufs=3)
        waof = waot[:, 0:KT * DA].rearrange("p (k d) -> p k d", d=DA)
        nc.gpsimd.dma_start(
            out=waof, in_=w_ao.rearrange("(kt p) d -> p kt d", p=P))
        waoh_t = zh_pool.tile([P, E], BF16, tag="zh")
        waoh = waoh_t[:, 0:KT * DA]
        nc.vector.tensor_copy(out=waoh, in_=waof.rearrange("p k d -> p (k d)"))
        tp = tp_ps()
        for k in range(KT):
            nc.tensor.matmul(tp[0:DA, k * P:(k + 1) * P],
                             waoh[:, k * DA:(k + 1) * DA], identh,
                             is_transpose=True)
        waoT = persist.tile([DA, KT, P], BF16, tag="waoT")
        nc.vector.tensor_copy(out=waoT,
                              in_=tp[0:DA, :].rearrange("p (k c) -> p k c", c=P))

        # w_base -> WbT[p, kt, n]
        WbT = persist.tile([P, KT, DM], BF16, tag="WbT")
        for nt in range(KT):
            wf = ldp.tile([P, E], F32, tag="wload", bufs=3)
            nc.gpsimd.dma_start(out=wf, in_=w_base[nt * P:(nt + 1) * P, :])
            wh = zh_pool.tile([P, E], BF16, tag="zh")
            nc.vector.tensor_copy(out=wh, in_=wf)
            tp = tp_ps()
            for k in range(KT):
                nc.tensor.matmul(tp[:, k * P:(k + 1) * P],
                                 wh[:, k * P:(k + 1) * P], identh,
                                 is_transpose=True)
            nc.vector.tensor_copy(
                out=WbT[:, :, nt * P:(nt + 1) * P],
                in_=tp.rearrange("p (k c) -> p k c", c=P))

        # wbsum[n] = sum_e w_base[n, e]
        wbs_sb = persist.tile([1, DM], BF16, tag="wbs_sb")
        for grp in range(2):
            wbs_ps = mm_ps()
            for k in range(KT):
                nc.tensor.matmul(wbs_ps[0:1, :], ones_col,
                                 WbT[:, k, grp * 512:(grp + 1) * 512],
                                 start=(k == 0), stop=(k == KT - 1))
            nc.vector.tensor_copy(out=wbs_sb[:, grp * 512:(grp + 1) * 512],
                                  in_=wbs_ps[0:1, :])

        # lora_up scaled
        lu_t = ldp.tile([P, E], F32, tag="wload", bufs=3)
        lu_f = lu_t[0:R, :]
        nc.gpsimd.dma_start(out=lu_f, in_=lu)
        lu8 = persist.tile([R, DM], BF16, tag="lu8")
        nc.scalar.mul(lu8, lu_f, SCALING)

        # lora_down [E, R] -> [p, kt, r]; gamma-folded + raw copies
        ld_t = ldp.tile([P, E], F32, tag="wload", bufs=3)
        ld_f = ld_t[:, 0:KT * R].rearrange("p (k r) -> p k r", r=R)
        nc.gpsimd.dma_start(out=ld_f, in_=ld.rearrange("(kt p) r -> p kt r", p=P))
        ld_raw = persist.tile([P, KT, R], BF16, tag="ld_raw")
        nc.vector.tensor_copy(out=ld_raw, in_=ld_f)
        ld_g = persist.tile([P, KT, R], BF16, tag="ld_g")
        for k in range(KT):
            nc.vector.tensor_scalar_mul(out=ld_g[:, k, :], in0=ld_f[:, k, :],
                                        scalar1=gT[:, k:k + 1])
        # bld[r] = sum_e beta[e] * lora_down[e, r]
        bld_ps = av_ps()
        for k in range(KT):
            nc.tensor.matmul(bld_ps[0:R, 0:1], ld_raw[:, k, :], bTh[:, k:k + 1],
                             start=(k == 0), stop=(k == KT - 1))
        bld = persist.tile([R, 1], F32, tag="bld")
        nc.vector.tensor_copy(out=bld, in_=bld_ps[0:R, 0:1])

        # qkv bias columns from beta (raw weights, pre-gamma-fold)
        bq0_ps = mm_ps()
        for k in range(KT):
            nc.tensor.matmul(bq0_ps[0:P, 0:1], wqkT[:, k, :],
                             bTh[:, k:k + 1], start=(k == 0), stop=(k == KT - 1))
        bq1_ps = mm_ps()
        for k in range(KT):
            nc.tensor.matmul(bq1_ps[0:DA, 0:1], wvT[:, k, :],
                             bTh[:, k:k + 1], start=(k == 0), stop=(k == KT - 1))
        bqk_col = persist.tile([P, 1], F32, tag="bqk_col")
        nc.vector.tensor_copy(out=bqk_col, in_=bq0_ps[0:P, 0:1])
        bv_col = persist.tile([DA, 1], F32, tag="bv_col")
        nc.vector.tensor_copy(out=bv_col, in_=bq1_ps[0:DA, 0:1])
        for k in range(KT):
            nc.vector.tensor_scalar_mul(out=wqkT[:, k, :], in0=wqkT[:, k, :],
                                        scalar1=gT[:, k:k + 1])
            nc.vector.tensor_scalar_mul(out=wvT[:, k, :], in0=wvT[:, k, :],
                                        scalar1=gT[:, k:k + 1])

        # ---------------- persistent activations ----------------
        zTg = [persist.tile([P, KT, 512], BF16, tag=f"zTg{g}", name=f"zTg{g}")
               for g in range(NG)]
        qT = persist.tile([DA, SOWN], BF16, tag="qT")
        kTt = persist.tile([DA, SFULL], BF16, tag="kTt")
        vT = persist.tile([DA, SFULL], BF16, tag="vT")
        v_nat = persist.tile([P, ST, DA], BF16, tag="v_nat")
        v_aug = persist.tile([P, NH, ST, HD + 1], BF16, tag="v_aug")
        aoTn = persist.tile([DA, SOWN], BF16, tag="aoTn")
        aopT = persist.tile([P, KT, SOWN], BF16, tag="aopT")
        t8 = persist.tile([R, SOWN], BF16, tag="t8")
        mu_row = persist.tile([1, SOWN], BF16, tag="mu_row")
        mu_all = persist.tile([P, MT], F32, tag="mu_all")
        std_all = persist.tile([P, MT], F32, tag="std_all")
        o_base = persist.tile([P, MT, DM], BF16, tag="o_base")

        nc.gpsimd.memset(v_aug[:, :, :, HD:HD + 1], 1.0)

        # ---------------- phase 1: layernorm + PE transposes ----------------
        for st in range(ST):
            own = st < MT
            if st in xfs:
                xf = xfs[st]
            else:
                xf = ldp.tile([P, E], F32, tag="xin", bufs=XPRE)
                eng = nc.gpsimd if st % 2 == 0 else nc.sync
                eng.dma_start(out=xf, in_=xsrc(st))

            stats = st_pool.tile([P, 2, 6], F32, tag="bnstats")
            xr = xf.rearrange("p (n f) -> p n f", f=512)
            for sg in range(2):
                nc.vector.bn_stats(out=stats[:, sg, :], in_=xr[:, sg, :])
            mv = st_pool.tile([P, 2], F32, tag="mv")
            nc.vector.bn_aggr(out=mv, in_=stats)

            rstd = st_pool.tile([P, 1], F32, tag="rstd")
            nc.scalar.activation(out=rstd, in_=mv[:, 1:2], func=AF.Sqrt,
                                 bias=eps_t)
            if own:
                nc.vector.tensor_copy(out=std_all[:, st:st + 1], in_=rstd)
                nc.vector.tensor_copy(out=mu_all[:, st:st + 1], in_=mv[:, 0:1])
            nc.vector.reciprocal(out=rstd, in_=rstd)
            nmr = st_pool.tile([P, 1], F32, tag="nmr")
            nc.vector.tensor_scalar(out=nmr, in0=mv[:, 0:1], scalar1=rstd,
                                    scalar2=-1.0, op0=MUL, op1=MUL)
            zh = zh_pool.tile([P, E], BF16, tag="zh")
            nc.scalar.activation(out=zh, in_=xf, func=AF.Identity,
                                 scale=rstd, bias=nmr)
            ztp = tp_ps()
            for k in range(KT):
                nc.tensor.matmul(ztp[:, k * P:(k + 1) * P],
                                 zh[:, k * P:(k + 1) * P], identh,
                                 is_transpose=True)
            g, sub = divmod(st, 4)
            nc.vector.tensor_copy(
                out=zTg[g][:, :, sub * P:(sub + 1) * P],
                in_=ztp.rearrange("p (k c) -> p k c", c=P))

        # mu_row[0, m] = mu[m]
        mu_allh = persist.tile([P, MT], BF16, tag="mu_allh")
        nc.vector.tensor_copy(out=mu_allh, in_=mu_all)
        for half in range(2):
            mur_ps = mm_ps()
            for mh in range(4):
                mt = half * 4 + mh
                nc.tensor.matmul(mur_ps[0:1, mh * P:(mh + 1) * P],
                                 mu_allh[:, mt:mt + 1], identh[:, 0:P],
                                 start=True, stop=True)
            nc.vector.tensor_copy(
                out=mu_row[:, half * 512:(half + 1) * 512],
                in_=mur_ps[0:1, :])

        # ---------------- phase 2: qkv projections ----------------
        for g in range(NG):
            pqk = mm_ps()
            for k in range(KT):
                nc.tensor.matmul(pqk, wqkT[:, k, :], zTg[g][:, k, :],
                                 start=(k == 0), stop=(k == KT - 1))
            nc.vector.tensor_scalar_add(
                out=kTt[:, g * 512:(g + 1) * 512],
                in0=pqk[DA:P, :], scalar1=bqk_col[DA:P, :])
            if g < 2:
                nc.vector.tensor_scalar_add(
                    out=qT[:, g * 512:(g + 1) * 512],
                    in0=pqk[0:DA, :], scalar1=bqk_col[0:DA, :])
        for g in range(NG):
            pv = mm_ps()
            for k in range(KT):
                nc.tensor.matmul(pv[0:DA, :], wvT[:, k, :], zTg[g][:, k, :],
                                 start=(k == 0), stop=(k == KT - 1))
            nc.vector.tensor_scalar_add(
                out=vT[:, g * 512:(g + 1) * 512],
                in0=pv[0:DA, :], scalar1=bv_col)
        # v natural via PE transposes: vT [64, jt*128] -> v_nat [p, jt, 64]
        vtp = tp_ps()
        for jt in range(ST):
            nc.tensor.matmul(vtp[:, jt * DA:(jt + 1) * DA],
                             vT[:, jt * P:(jt + 1) * P], identh[0:DA, 0:DA],
                             is_transpose=True)
        nc.vector.tensor_copy(out=v_nat,
                              in_=vtp.rearrange("p (j d) -> p j d", d=DA))
        nc.vector.tensor_copy(out=v_aug[:, 0, :, 0:HD], in_=v_nat[:, :, 0:HD])
        nc.vector.tensor_copy(out=v_aug[:, 1, :, 0:HD], in_=v_nat[:, :, HD:DA])

        # ---------------- phase 3: attention ----------------
        # base matmul emitter: out_base = std[m]*(z@Wb^T) in bf16; these
        # matmuls are independent of attention and fill PE gaps while the
        # ACT engine works through the softmax exps.
        def emit_base(mt):
            g, sub = divmod(mt, 4)
            for grp in range(2):
                pA = mm_ps()
                for k in range(KT):
                    nc.tensor.matmul(pA,
                                     zTg[g][:, k, sub * P:(sub + 1) * P],
                                     WbT[:, k, grp * 512:(grp + 1) * 512],
                                     start=(k == 0), stop=(k == KT - 1))
                nc.vector.tensor_scalar_mul(
                    out=o_base[:, mt, grp * 512:(grp + 1) * 512],
                    in0=pA, scalar1=std_all[:, mt:mt + 1])

        CH = 8  # jt per chunk
        base_mt = [0]
        for h in range(NH):
            d0 = h * HD
            pav = [av_ps() for _ in range(2)]
            for chunk in range(ST // CH):
                pTc = pt_pool.tile([P, CH, SOWN], BF16, tag="pT")
                for j8 in range(CH):
                    jt = chunk * CH + j8
                    for mgrp in range(2):
                        psc = sc_ps()
                        nc.tensor.matmul(
                            psc,
                            kTt[d0:d0 + HD, jt * P:(jt + 1) * P],
                            qT[d0:d0 + HD, mgrp * 512:(mgrp + 1) * 512],
                            start=True, stop=True)
                        nc.scalar.activation(
                            out=pTc[:, j8, mgrp * 512:(mgrp + 1) * 512],
                            in_=psc, func=AF.Exp, scale=ATT_SCALE)
                for _ in range(2):
                    emit_base(base_mt[0]); base_mt[0] += 1
                for j8 in range(CH):
                    jt = chunk * CH + j8
                    for mgrp in range(2):
                        nc.tensor.matmul(
                            pav[mgrp], v_aug[:, h, jt, :],
                            pTc[:, j8, mgrp * 512:(mgrp + 1) * 512],
                            start=(jt == 0), stop=(jt == ST - 1))
            for mgrp in range(2):
                rr = st_pool.tile([1, 512], F32, tag="rr", bufs=2)
                nc.vector.reciprocal(out=rr, in_=pav[mgrp][HD:HD + 1, :])
                rrh = st_pool.tile([1, 512], BF16, tag="rrh", bufs=2)
                nc.vector.tensor_copy(out=rrh, in_=rr)
                rrb_ps = av_ps()
                nc.tensor.matmul(rrb_ps[0:HD, :], ones_row[0:1, 0:HD],
                                 rrh, start=True, stop=True)
                rrb_sb = st_pool.tile([HD, 512], F32, tag="rrb_sb", bufs=2)
                nc.vector.tensor_copy(out=rrb_sb, in_=rrb_ps[0:HD, :])
                nc.vector.tensor_tensor(
                    out=aoTn[d0:d0 + HD, mgrp * 512:(mgrp + 1) * 512],
                    in0=pav[mgrp][0:HD, :], in1=rrb_sb, op=MUL)

        # ---------------- phase 4: attn_out projection ----------------
        for et in range(KT):
            for mgrp in range(2):
                pp = mm_ps()
                nc.tensor.matmul(pp, waoT[:, et, :],
                                 aoTn[:, mgrp * 512:(mgrp + 1) * 512],
                                 start=True, stop=True)
                nc.vector.tensor_copy(
                    out=aopT[:, et, mgrp * 512:(mgrp + 1) * 512], in_=pp)

        # ---------------- phase 5: lora down ----------------
        for mgrp in range(2):
            p5 = av_ps()
            for k in range(KT):
                nc.tensor.matmul(p5[0:R, :], ld_g[:, k, :], zTg[mgrp][:, k, :],
                                 start=(k == 0), stop=False)
            for k in range(KT):
                nc.tensor.matmul(p5[0:R, :], ld_raw[:, k, :],
                                 aopT[:, k, mgrp * 512:(mgrp + 1) * 512],
                                 start=False, stop=(k == KT - 1))
            nc.vector.tensor_scalar_add(
                out=t8[:, mgrp * 512:(mgrp + 1) * 512],
                in0=p5[0:R, :], scalar1=bld)

        # ---------------- phase 6 tail: lora up + mu correction + store ----------------
        # out = o_base + (t@lu/8 + mu x wbsum)
        for mt in range(MT):
            o_t = o_pool.tile([P, DM], F32, tag="o_t")
            for grp in range(2):
                pB = mm_ps()
                nc.tensor.matmul(pB, t8[:, mt * P:(mt + 1) * P],
                                 lu8[:, grp * 512:(grp + 1) * 512],
                                 start=True, stop=False)
                nc.tensor.matmul(pB, mu_row[:, mt * P:(mt + 1) * P],
                                 wbs_sb[:, grp * 512:(grp + 1) * 512],
                                 start=False, stop=True)
                nc.vector.tensor_tensor(
                    out=o_t[:, grp * 512:(grp + 1) * 512],
                    in0=o_base[:, mt, grp * 512:(grp + 1) * 512],
                    in1=pB, op=mybir.AluOpType.add)
                eng = nc.sync if grp == 0 else nc.gpsimd
                eng.dma_start(
                    out=out_d[mt * P:(mt + 1) * P, grp * 512:(grp + 1) * 512],
                    in_=o_t[:, grp * 512:(grp + 1) * 512])

    nc.compile()
    return nc


_NC_CACHE = None


def _get_nc():
    global _NC_CACHE
    if _NC_CACHE is None:
        _NC_CACHE = build_kernel()
    return _NC_CACHE


def kernel(x, w_base, ln_gamma, ln_beta, lora_down, lora_up, w_qkv, w_attn_out,
           _trace=False):
    x = np.ascontiguousarray(np.asarray(x, dtype=np.float32))
    wk = {
        "w_base": np.ascontiguousarray(np.asarray(w_base, np.float32)),
        "ln_g": np.ascontiguousarray(np.asarray(ln_gamma, np.float32)),
        "ln_b": np.ascontiguousarray(np.asarray(ln_beta, np.float32)),
        "ld": np.ascontiguousarray(np.asarray(lora_down, np.float32)),
        "lu": np.ascontiguousarray(np.asarray(lora_up, np.float32)),
        "w_qkv": np.ascontiguousarray(np.asarray(w_qkv, np.float32)),
        "w_ao": np.ascontiguousarray(np.asarray(w_attn_out, np.float32)),
    }
    nc = _get_nc()
    in_maps = []
    for c in range(NC):
        b, half = divmod(c, 2)
        own = np.ascontiguousarray(x[b, half * SOWN:(half + 1) * SOWN])
        oth = np.ascontiguousarray(x[b, (1 - half) * SOWN:(2 - half) * SOWN])
        in_maps.append({"x_own": own, "x_oth": oth, **wk})
    res = run_bass_kernel_spmd(nc, in_maps, core_ids=list(range(NC)), trace=_trace)
    B, S = x.shape[0], x.shape[1]
    out = np.empty((B, S, DM), np.float32)
    for c in range(NC):
        b, half = divmod(c, 2)
        out[b, half * SOWN:(half + 1) * SOWN] = res.results[c]["out"]
    if _trace:
        kernel.last_exec_time_ns = res.exec_time_ns
        kernel.last_results = res
    return out
